# revision 1
# baseline (speedup 1.0000x reference)
"""TRN2 Bass kernel for nn_DecoderLayer_47175920779446.

Full decoder layer: qkv (mul-bias) -> 16-head attention -> +res -> LN ->
FFN(relu, mul-bias) -> +res -> LN, on x[2, 2048, 1024] fp32.

Sharding (8 cores): attention is sharded by (batch, 4 heads): core c handles
batch c//4, heads 4*(c%4)..4*(c%4)+3 over all 2048 tokens of its batch.
An 8-core AllToAll reshards attention output to token sharding (512 tokens
per core), under which LN1/FFN/LN2 run with fully replicated weights.

Precision: scores need ~fp32 accuracy (std ~256 feeding exp): q,k chain runs
fp32r (11-bit mantissa) projections, then an exact bf16 hi/lo split with a
2-matmul scheme: S = qh*kh + m_hat (main, K=65 with a fused bias row) plus
[qh;ql]*[kl;kh] (cross, K=128). V/P/FFN run bf16; residuals/LN run fp32.
"""
import contextlib
import numpy as np
import ml_dtypes

import concourse.bass as bass
import concourse.tile as tile
from concourse import bacc, mybir
from concourse.bass_utils import run_bass_kernel_spmd
from concourse.bass_interp import get_hw_module
from concourse.masks import make_identity

H, NH, HD, FF = 1024, 16, 64, 4096
B, T = 2, 2048
EPS = 1e-6
NCORES = 8
HPC = NH // 4          # 4 heads per core
TOK = (B * T) // NCORES  # 512 tokens per core
NKC = T // 128         # 16 key chunks
NG = T // 512          # 4 query groups
KCH = H // 128         # 8 contraction chunks for qkv
f32, f32r, bf16 = mybir.dt.float32, mybir.dt.float32r, mybir.dt.bfloat16
AF = mybir.ActivationFunctionType
ALU = mybir.AluOpType


def _round_mant(x, bits=11):
    xi = np.ascontiguousarray(x, np.float32).view(np.int32)
    shift = 23 - bits
    bias = (1 << (shift - 1)) - 1 + ((xi >> shift) & 1)
    xi = (xi + bias) & ~((1 << shift) - 1)
    return xi.view(np.float32)


def _build_program(sim_single=False):
    nc = bacc.Bacc("TRN2", target_bir_lowering=False, debug=False,
                   num_devices=1 if sim_single else NCORES)
    ap = {}
    ap["xT"] = nc.dram_tensor("xT", [H, T], f32r, kind="ExternalInput").ap()
    ap["xres"] = nc.dram_tensor("xres", [TOK, H], f32, kind="ExternalInput").ap()
    for w in ("wq", "wk", "wv"):
        ap[w] = nc.dram_tensor(w, [H, 4 * HD], f32r, kind="ExternalInput").ap()
    ap["w1"] = nc.dram_tensor("w1", [H, FF], bf16, kind="ExternalInput").ap()
    ap["w2"] = nc.dram_tensor("w2", [FF, H], bf16, kind="ExternalInput").ap()
    ap["lnw"] = nc.dram_tensor("lnw", [4, H], f32, kind="ExternalInput").ap()
    ap["bsel"] = nc.dram_tensor("bsel", [2], f32, kind="ExternalInput").ap()
    out_ap = nc.dram_tensor("out", [TOK, H], f32, kind="ExternalOutput").ap()

    with tile.TileContext(nc) as tc:
        ctx = contextlib.ExitStack()
        with ctx:
            const = ctx.enter_context(tc.tile_pool(name="const", bufs=1))
            dram = ctx.enter_context(tc.tile_pool(name="dram", bufs=1, space="DRAM"))

            ident = const.tile([128, 128], f32)
            make_identity(nc, ident[:])
            bs = const.tile([128, 2], f32)
            nc.sync.dma_start(bs[:], ap["bsel"].partition_broadcast(128))

            w1p = ctx.enter_context(tc.tile_pool(name="w1p", bufs=2))
            a2a_in = dram.tile([NCORES, TOK, 4 * HD], f32)
            a2a_out = dram.tile([NCORES, TOK, 4 * HD], f32)

            # ---------------- attention scope ----------------
            actx = contextlib.ExitStack()
            with actx:
                wpool = actx.enter_context(tc.tile_pool(name="wpool", bufs=1))
                qk = actx.enter_context(tc.tile_pool(name="qk", bufs=1))
                xgp = actx.enter_context(tc.tile_pool(name="xgp", bufs=4))
                sb = actx.enter_context(tc.tile_pool(name="sb", bufs=3))
                small = actx.enter_context(tc.tile_pool(name="small", bufs=4))
                psn = actx.enter_context(
                    tc.tile_pool(name="psn", bufs=2, space="PSUM"))
                pss = actx.enter_context(
                    tc.tile_pool(name="pss", bufs=2, space="PSUM"))
                pso = actx.enter_context(
                    tc.tile_pool(name="pso", bufs=1, space="PSUM"))
                psm = actx.enter_context(
                    tc.tile_pool(name="psm", bufs=1, space="PSUM"))

                w_sb = {}
                for w in ("wq", "wk", "wv"):
                    w_sb[w] = wpool.tile([128, KCH, 4 * HD], f32r, name=f"sb_{w}")
                    nc.sync.dma_start(
                        w_sb[w][:], ap[w].rearrange("(a p) c -> p a c", p=128))

                # per-head score operands
                til_q, til_k, cr_q, cr_k = {}, {}, {}, {}
                for h in range(HPC):
                    til_q[h] = qk.tile([65, T], bf16, name=f"til_q{h}", tag="tq", bufs=HPC)
                    til_k[h] = qk.tile([65, T], bf16, name=f"til_k{h}", tag="tk", bufs=HPC)
                    cr_q[h] = qk.tile([128, T], bf16, name=f"cr_q{h}", tag="cq", bufs=HPC)
                    cr_k[h] = qk.tile([128, T], bf16, name=f"cr_k{h}", tag="ck", bufs=HPC)
                    nc.gpsimd.memset(til_k[h][64:65, :], 1.0)
                vn = []
                for kc in range(NKC):
                    v = qk.tile([128, HPC, 65], bf16, name=f"vn{kc}", tag="vn", bufs=NKC)
                    nc.gpsimd.memset(v[:, :, 64:65], 1.0)
                    vn.append(v)

                # ---- QKV projection: all xg resident; K for all groups
                # first so attention's stage_a can begin ~18us earlier ----
                xgs = []
                for g in range(NG):
                    gsl = slice(512 * g, 512 * (g + 1))
                    xg = xgp.tile([128, KCH, 512], f32r, name=f"xg{g}", tag="xg", bufs=4)
                    nc.sync.dma_start(
                        xg[:], ap["xT"].rearrange("(a p) t -> p a t", p=128)[:, :, gsl])
                    xgs.append(xg)

                def proj_pass(name, til, cr, g):
                    gsl = slice(512 * g, 512 * (g + 1))
                    for hp in range(2):  # head pairs
                        p = pss.tile([128, 512], f32, tag="st", name="pqk")
                        for a in range(KCH):
                            nc.tensor.matmul(
                                p[:], w_sb[name][:, a, 128 * hp:128 * (hp + 1)],
                                xgs[g][:, a, :], start=(a == 0), stop=(a == KCH - 1))
                        for hl in range(2):
                            h = 2 * hp + hl
                            rows = slice(64 * hl, 64 * (hl + 1))
                            nc.scalar.activation(til[h][0:64, gsl], p[rows, :], AF.Copy)
                            if name == "wq":
                                hi_rows, lo_rows = slice(0, 64), slice(64, 128)
                            else:
                                hi_rows, lo_rows = slice(64, 128), slice(0, 64)
                            nc.sync.dma_start(cr[h][hi_rows, gsl], til[h][0:64, gsl])
                            nc.vector.scalar_tensor_tensor(
                                out=cr[h][lo_rows, gsl], in0=p[rows, :], scalar=1.0,
                                in1=til[h][0:64, gsl], op0=ALU.mult, op1=ALU.subtract)

                for g in range(NG):
                    proj_pass("wk", til_k, cr_k, g)
                for g in range(NG):
                    proj_pass("wq", til_q, cr_q, g)
                    for tt in range(4):  # V natural per token tile
                        kc = 4 * g + tt
                        p = pss.tile([128, 4 * HD], f32, tag="st", name="pv")
                        for a in range(KCH):
                            nc.tensor.matmul(
                                p[:], xgs[g][:, a, 128 * tt:128 * (tt + 1)],
                                w_sb["wv"][:, a, :], start=(a == 0), stop=(a == KCH - 1))
                        nc.scalar.activation(
                            vn[kc][:, :, 0:64],
                            p[:].rearrange("p (h d) -> p h d", h=HPC), AF.Copy)

                # ---- attention, software-pipelined over (head, group) units ----
                # stage A(unit): natural-S -> row max -> m_hat_neg row (PE+DVE)
                # stage B(unit): S~^T -> exp -> PV -> O out (PE+ACT+DVE)
                # emit A(i+2) between B(i-1) and B(i) so the m_hat chain is
                # hidden under two unit periods of PE work.
                units = [(h, g) for h in range(HPC) for g in range(NG)]

                def stage_a1(h, g):
                    # natural-S matmuls + DVE max reduces (no PE dependency on DVE)
                    mstage = small.tile([128, 4], f32, tag="mstage", name="mstage", bufs=2)
                    for qt in range(4):
                        qsl = slice(512 * g + 128 * qt, 512 * g + 128 * (qt + 1))
                        negmax = []
                        for half in range(2):
                            sn = psn.tile([128, 1024], f32, name="sn")
                            for j in range(2):
                                ks = slice(1024 * half + 512 * j,
                                           1024 * half + 512 * (j + 1))
                                nc.tensor.matmul(
                                    sn[:, 512 * j:512 * (j + 1)],
                                    til_q[h][0:64, qsl], til_k[h][0:64, ks],
                                    start=True, stop=True)
                            nm = small.tile([128, 1], f32, tag="nm", name="nm")
                            nc.vector.tensor_reduce(
                                nm[:], sn[:], axis=mybir.AxisListType.X,
                                op=ALU.max, negate=True)
                            negmax.append(nm)
                        nc.vector.tensor_tensor(
                            mstage[:, qt:qt + 1], negmax[0][:], negmax[1][:], ALU.min)
                    return mstage

                def stage_a2(h, g, mstage):
                    # emitted a period later so the PE transpose never waits on DVE
                    for qt in range(4):
                        qsl = slice(512 * g + 128 * qt, 512 * g + 128 * (qt + 1))
                        mt = psm.tile([1, 128], f32, tag="mt", name="mt")
                        nc.tensor.transpose(mt[:], mstage[:, qt:qt + 1], ident[:])
                        nc.vector.tensor_copy(til_q[h][64:65, qsl], mt[:])

                def stage_b(h, g):
                    gsl = slice(512 * g, 512 * (g + 1))
                    o_acc = pso.tile([65, 512], f32, name="o_acc")
                    pts = {}
                    PVLAG = 2

                    def pv(kc):
                        nc.tensor.matmul(o_acc[:], vn[kc][:, h, :], pts.pop(kc)[:],
                                         start=(kc == 0), stop=(kc == NKC - 1))

                    for kc in range(NKC):
                        ksl = slice(128 * kc, 128 * (kc + 1))
                        st = pss.tile([128, 512], f32, tag="st", name="st")
                        nc.tensor.matmul(st[:], til_k[h][0:65, ksl],
                                         til_q[h][0:65, gsl], start=True, stop=False)
                        nc.tensor.matmul(st[:], cr_k[h][:, ksl],
                                         cr_q[h][:, gsl], start=False, stop=True)
                        pt = sb.tile([128, 512], bf16, tag="pt", name="pt", bufs=6)
                        nc.scalar.activation(pt[:], st[:], AF.Exp)
                        pts[kc] = pt
                        if kc >= PVLAG:
                            pv(kc - PVLAG)
                    for kc in range(NKC - PVLAG, NKC):
                        pv(kc)
                    ot = sb.tile([65, 512], f32, tag="ot", name="ot")
                    nc.scalar.activation(ot[:], o_acc[:], AF.Copy)
                    # transpose to natural, scale by 1/denom, ship to a2a_in
                    for tt in range(4):
                        qt = 4 * g + tt
                        op_ = psm.tile([128, 65], f32, tag="mt", name="opt")
                        nc.tensor.transpose(
                            op_[:], ot[0:65, 128 * tt:128 * (tt + 1)],
                            ident[0:65, 0:65])
                        rc = small.tile([128, 1], f32, tag="rc", name="rc")
                        nc.vector.reciprocal(rc[:], op_[:, 64:65])
                        ob = sb.tile([128, HD], f32, tag="ob", name="ob", bufs=4)
                        nc.vector.tensor_scalar_mul(ob[:], op_[:, 0:64], rc[:])
                        j, r = qt // 4, qt % 4
                        for blk in (j, j + 4):
                            nc.sync.dma_start(
                                a2a_in[blk, 128 * r:128 * (r + 1),
                                       64 * h:64 * (h + 1)], ob[:])

                LOOKAHEAD = 3
                mstages, done_a2 = {}, set()
                for k in range(min(LOOKAHEAD, len(units))):
                    mstages[k] = stage_a1(*units[k])
                    stage_a2(*units[k], mstages.pop(k))
                    done_a2.add(k)
                for i, (h, g) in enumerate(units):
                    j = i + LOOKAHEAD
                    if j < len(units):
                        mstages[j] = stage_a1(*units[j])
                    j2 = i + LOOKAHEAD - 1
                    if j2 < len(units) and j2 not in done_a2:
                        stage_a2(*units[j2], mstages.pop(j2))
                        done_a2.add(j2)
                    stage_b(h, g)

            if sim_single:
                # timing stand-in for the 4MB AllToAll
                nc.sync.dma_start(a2a_out[:], a2a_in[:])
            else:
                nc.gpsimd.collective_compute(
                    "AllToAll", ALU.bypass,
                    replica_groups=[list(range(NCORES))],
                    ins=[a2a_in.opt()], outs=[a2a_out.opt()])

            # ---------------- FFN / LN scope ----------------
            fctx = contextlib.ExitStack()
            with fctx:
                fsb = fctx.enter_context(tc.tile_pool(name="fsb", bufs=2))
                o1p = fctx.enter_context(tc.tile_pool(name="o1p", bufs=1))
                w2p = fctx.enter_context(tc.tile_pool(name="w2p", bufs=1))
                fsm = fctx.enter_context(tc.tile_pool(name="fsm", bufs=4))
                psf = fctx.enter_context(
                    tc.tile_pool(name="psf", bufs=2, space="PSUM"))
                psg = fctx.enter_context(
                    tc.tile_pool(name="psg", bufs=2, space="PSUM"))

                lnbc = {}
                for i, nm in enumerate(("g1", "b1", "g2", "b2")):
                    lnbc[nm] = o1p.tile([128, H], f32, name=f"ln_{nm}", tag="lnbc", bufs=4)
                    nc.sync.dma_start(
                        lnbc[nm][:], ap["lnw"][i, :].partition_broadcast(128))

                out1 = o1p.tile([128, 4, H], f32, name="out1")       # natural, fp32
                out1T = o1p.tile([128, KCH, 512], bf16, name="out1T")  # transposed
                ht = o1p.tile([128, FF // 128, 512], bf16, name="ht")

                def layer_norm_to(dst, src, g_bc, b_bc, work):
                    """dst = gamma*(src-mean)/(std_unbiased+EPS)+beta, [128,H] tiles."""
                    stats = fsm.tile([128, 2, 6], f32, tag="stats", name="stats")
                    for hf in range(2):
                        nc.vector.bn_stats(stats[:, hf, :],
                                           src[:, 512 * hf:512 * (hf + 1)])
                    mv = fsm.tile([128, 2], f32, tag="mv", name="mv")
                    nc.vector.bn_aggr(mv[:], stats[:])
                    sd = fsm.tile([128, 1], f32, tag="sd", name="sd")
                    nc.scalar.activation(sd[:], mv[:, 1:2], AF.Sqrt,
                                         scale=float(H) / (H - 1))
                    nc.vector.tensor_scalar_add(sd[:], sd[:], EPS)
                    rs = fsm.tile([128, 1], f32, tag="rs", name="rs")
                    nc.vector.reciprocal(rs[:], sd[:])
                    nc.vector.tensor_scalar(out=work[:], in0=src[:],
                                            scalar1=mv[:, 0:1], scalar2=rs[:],
                                            op0=ALU.subtract, op1=ALU.mult)
                    nc.vector.tensor_mul(work[:], work[:], g_bc[:])
                    nc.vector.tensor_add(dst[:], work[:], b_bc[:])

                # LN1 over x + attn, per token tile
                for tt in range(4):
                    tsl = slice(128 * tt, 128 * (tt + 1))
                    at = fsb.tile([128, H], f32, tag="ta", name="at")
                    bt = fsb.tile([128, H], f32, tag="tb", name="bt")
                    for sl in range(4):
                        csl = slice(256 * sl, 256 * (sl + 1))
                        nc.sync.dma_start(at[:, csl], a2a_out[sl, tsl, :])
                        nc.sync.dma_start(bt[:, csl], a2a_out[sl + 4, tsl, :])
                    xt = fsb.tile([128, H], f32, tag="tc", name="xt")
                    nc.sync.dma_start(xt[:], ap["xres"][tsl, :])
                    nc.vector.tensor_scalar_mul(at[:], at[:], bs[:, 0:1])
                    nc.vector.scalar_tensor_tensor(
                        out=at[:], in0=bt[:], scalar=bs[:, 1:2], in1=at[:],
                        op0=ALU.mult, op1=ALU.add)
                    nc.vector.tensor_add(at[:], at[:], xt[:])
                    wk_ = fsb.tile([128, H], f32, tag="td", name="wk_")
                    layer_norm_to(out1[:, tt, :], at, lnbc["g1"], lnbc["b1"], wk_)
                    # transpose out1 tile -> out1T
                    for a in range(KCH):
                        tp = psg.tile([128, 128], f32, tag="tp", name="tp", bufs=2)
                        nc.tensor.transpose(
                            tp[:], out1[:, tt, 128 * a:128 * (a + 1)], ident[:])
                        nc.scalar.activation(
                            out1T[:, a, 128 * tt:128 * (tt + 1)], tp[:], AF.Copy)

                # FFN1: ht[f, t] = relu(W1^T x out1T), f-major
                for fb in range(KCH):  # 8 blocks of 512 ff cols
                    w1t = w1p.tile([128, KCH, 512], bf16, name="w1t")
                    nc.sync.dma_start(
                        w1t[:], ap["w1"].rearrange("(a p) f -> p a f", p=128)
                        [:, :, 512 * fb:512 * (fb + 1)])
                    for fq in range(4):  # 4 x 128 f-rows per block
                        ft = 4 * fb + fq
                        hp_ = psf.tile([128, 512], f32, tag="hp", name="hp", bufs=3)
                        for a in range(KCH):
                            nc.tensor.matmul(
                                hp_[:], w1t[:, a, 128 * fq:128 * (fq + 1)],
                                out1T[:, a, :], start=(a == 0), stop=(a == KCH - 1))
                        nc.scalar.activation(ht[:, ft, :], hp_[:], AF.Relu)

                # FFN2 token-major (both W2 halves resident) + fused LN2 tail
                f2 = o1p.tile([128, 4, H], f32, name="f2")
                w2ts = []
                for oc in range(2):
                    w2t = w2p.tile([128, FF // 128, 512], bf16, name=f"w2t{oc}",
                                   tag="w2t", bufs=2)
                    nc.sync.dma_start(
                        w2t[:], ap["w2"].rearrange("(a p) o -> p a o", p=128)
                        [:, :, 512 * oc:512 * (oc + 1)])
                    w2ts.append(w2t)
                for tt in range(4):
                    for oc in range(2):
                        acc = psf.tile([128, 512], f32, tag="o2", name="o2acc")
                        for ft in range(FF // 128):
                            nc.tensor.matmul(
                                acc[:], ht[:, ft, 128 * tt:128 * (tt + 1)],
                                w2ts[oc][:, ft, :], start=(ft == 0),
                                stop=(ft == FF // 128 - 1))
                        nc.scalar.activation(
                            f2[:, tt, 512 * oc:512 * (oc + 1)], acc[:], AF.Copy)
                    h2 = fsb.tile([128, H], f32, tag="ta", name="h2")
                    nc.vector.tensor_add(h2[:], out1[:, tt, :], f2[:, tt, :])
                    fin = fsb.tile([128, H], f32, tag="tb", name="fin")
                    wk2 = fsb.tile([128, H], f32, tag="tc", name="wk2")
                    layer_norm_to(fin, h2, lnbc["g2"], lnbc["b2"], wk2)
                    nc.sync.dma_start(out_ap[128 * tt:128 * (tt + 1), :], fin[:])

    nc.compile()
    if not sim_single:
        nc.m = get_hw_module(nc.m)
    return nc


_NC_CACHE = {}


def _get_program():
    if "nc" not in _NC_CACHE:
        _NC_CACHE["nc"] = _build_program()
    return _NC_CACHE["nc"]


def _prep_inputs(x, Wqkv, bqkv, W1, b1, W2, b2, gamma1, beta1, gamma2, beta2):
    """Host-side slicing/folding into per-core in_maps."""
    x = np.asarray(x, np.float32)
    Wqkv = np.asarray(Wqkv, np.float32)
    bqkv = np.asarray(bqkv, np.float32)
    d = np.arange(HD)
    hh = np.arange(NH)
    # qkv reshape in reference: [B,T,HD,3,NH] -> col = d*48 + k*16 + h
    cols = d[:, None, None] * (3 * NH) + np.arange(3)[None, :, None] * NH \
        + hh[None, None, :]
    Wq = Wqkv[:, cols[:, 0, :]] * (bqkv[cols[:, 0, :]] / np.sqrt(H))[None]
    Wk = Wqkv[:, cols[:, 1, :]] * bqkv[cols[:, 1, :]][None]
    Wv = Wqkv[:, cols[:, 2, :]] * bqkv[cols[:, 2, :]][None]
    # -> [H, HD, NH]; per-core head-major layout [H, 4*HD] (head-local major)
    Wq = np.transpose(Wq, (0, 2, 1))  # [H, NH, HD]
    Wk = np.transpose(Wk, (0, 2, 1))
    Wv = np.transpose(Wv, (0, 2, 1))
    W1e = (np.asarray(W1, np.float32) * np.asarray(b1, np.float32)[None]) \
        .astype(ml_dtypes.bfloat16)
    W2e = (np.asarray(W2, np.float32) * np.asarray(b2, np.float32)[None]) \
        .astype(ml_dtypes.bfloat16)
    lnw = np.stack([gamma1, beta1, gamma2, beta2]).astype(np.float32)
    xT = [_round_mant(np.ascontiguousarray(x[b].T)) for b in range(B)]
    in_maps = []
    for c in range(NCORES):
        b, grp = c // 4, c % 4
        heads = slice(4 * grp, 4 * grp + 4)
        in_maps.append({
            "xT": xT[b],
            "xres": np.ascontiguousarray(x[b, 512 * grp:512 * (grp + 1), :]),
            "wq": _round_mant(Wq[:, heads, :].reshape(H, 4 * HD)),
            "wk": _round_mant(Wk[:, heads, :].reshape(H, 4 * HD)),
            "wv": _round_mant(Wv[:, heads, :].reshape(H, 4 * HD)),
            "w1": W1e, "w2": W2e, "lnw": lnw,
            "bsel": np.array([1.0, 0.0] if b == 0 else [0.0, 1.0], np.float32),
        })
    return in_maps


def kernel(x, Wqkv, bqkv, W1, b1, W2, b2, gamma1, beta1, gamma2, beta2,
           _trace=False):
    nc = _get_program()
    in_maps = _prep_inputs(x, Wqkv, bqkv, W1, b1, W2, b2,
                           gamma1, beta1, gamma2, beta2)
    res = run_bass_kernel_spmd(nc, in_maps, core_ids=list(range(NCORES)),
                               trace=_trace)
    out = np.stack([res.results[c]["out"] for c in range(NCORES)])
    out = out.reshape(B, T, H).astype(np.float32)
    if _trace:
        kernel.last_results = res
    return out



# revision 53
# speedup vs baseline: 1.0904x; 1.0904x over previous
"""TRN2 Bass kernel for nn_DecoderLayer_47175920779446.

Full decoder layer: qkv (mul-bias) -> 16-head attention -> +res -> LN ->
FFN(relu, mul-bias) -> +res -> LN, on x[2, 2048, 1024] fp32.

Sharding (8 cores): attention is sharded by (batch, 4 heads): core c handles
batch c//4, heads 4*(c%4)..4*(c%4)+3 over all 2048 tokens of its batch.
Four quarter-AllToAlls (bf16) reshard attention output to strided token
sharding: core c owns tokens {512q + 128*(c%4) + p} of batch c//4, so each
quarter fires right after its sender token-group finishes and LN1 overlaps
the remaining attention. Cross-batch blocks are zeroed at the sender (scaled
by the per-core bsel mask) so receivers just add both halves.

Precision: scores need ~fp32 accuracy (std ~256 feeding exp): q,k chain runs
fp16 projections (11-bit mantissa operands, fp32 accum), then an exact bf16
hi/lo split with a 2-matmul scheme: S = qh*kh + m_hat (main, K=65 with a
fused bias row) plus [qh;ql]*[kl;kh] (cross, K=128). The row-max pass (only
needs ~+-50 accuracy) runs fp8e4m3 in DoubleRow perf mode (2 K-tiles/pass).
V/P/FFN run bf16; LN chains run bf16 (2x DVE); residual adds keep fp32
accumulation in PSUM.
"""
import contextlib
import numpy as np
import ml_dtypes

import concourse.bass as bass
import concourse.tile as tile
from concourse import bacc, mybir
from concourse.bass_utils import run_bass_kernel_spmd
from concourse.bass_interp import get_hw_module
from concourse.masks import make_identity

H, NH, HD, FF = 1024, 16, 64, 4096
B, T = 2, 2048
EPS = 1e-6
NCORES = 8
HPC = NH // 4          # 4 heads per core
TOK = (B * T) // NCORES  # 512 tokens per core
NKC = T // 128         # 16 key chunks
NG = T // 512          # 4 query groups
KCH = H // 128         # 8 contraction chunks for qkv
f32, f32r, bf16 = mybir.dt.float32, mybir.dt.float32r, mybir.dt.bfloat16
fp8 = mybir.dt.float8e4
AF = mybir.ActivationFunctionType
ALU = mybir.AluOpType
DR = mybir.MatmulPerfMode.DoubleRow


def _round_mant(x, bits=11):
    xi = np.ascontiguousarray(x, np.float32).view(np.int32)
    shift = 23 - bits
    bias = (1 << (shift - 1)) - 1 + ((xi >> shift) & 1)
    xi = (xi + bias) & ~((1 << shift) - 1)
    return xi.view(np.float32)


def _build_program(sim_single=False):
    nc = bacc.Bacc("TRN2", target_bir_lowering=False, debug=False,
                   num_devices=1 if sim_single else NCORES)
    ap = {}
    ap["xT"] = nc.dram_tensor("xT", [H, T], f32r, kind="ExternalInput").ap()
    ap["xres"] = nc.dram_tensor("xres", [TOK, H], bf16, kind="ExternalInput").ap()
    for w in ("wq", "wk", "wv"):
        ap[w] = nc.dram_tensor(w, [H, 4 * HD], f32r, kind="ExternalInput").ap()
    ap["w1"] = nc.dram_tensor("w1", [H, FF], bf16, kind="ExternalInput").ap()
    ap["w2"] = nc.dram_tensor("w2", [FF, H], bf16, kind="ExternalInput").ap()
    ap["lnw"] = nc.dram_tensor("lnw", [4, H], bf16, kind="ExternalInput").ap()
    ap["bsel"] = nc.dram_tensor("bsel", [2], f32, kind="ExternalInput").ap()
    out_ap = nc.dram_tensor("out", [TOK, H], bf16, kind="ExternalOutput").ap()

    with tile.TileContext(nc) as tc:
        ctx = contextlib.ExitStack()
        with ctx:
            const = ctx.enter_context(tc.tile_pool(name="const", bufs=1))
            dram = ctx.enter_context(tc.tile_pool(name="dram", bufs=1, space="DRAM"))
            pre = ctx.enter_context(tc.tile_pool(name="pre", bufs=1))
            w1p = ctx.enter_context(tc.tile_pool(name="w1p", bufs=2))
            fsb = ctx.enter_context(tc.tile_pool(name="fsb", bufs=1))
            fsm = ctx.enter_context(tc.tile_pool(name="fsm", bufs=4))
            o1p = ctx.enter_context(tc.tile_pool(name="o1p", bufs=1))

            # quarter a2a buffers: block d = [128 tokens, 256 head-cols]
            a2a_in = [dram.tile([NCORES, 128, 4 * HD], bf16, name=f"a2ai{q}")
                      for q in range(NG)]
            a2a_out = [dram.tile([NCORES, 128, 4 * HD], bf16, name=f"a2ao{q}")
                       for q in range(NG)]

            out1 = o1p.tile([128, 4, H], bf16, name="out1")      # natural
            out1T = o1p.tile([128, KCH, 512], bf16, name="out1T")  # transposed

            lnbc = {}
            for i, nm in enumerate(("g1", "b1", "g2", "b2")):
                lnbc[nm] = pre.tile([128, H], bf16, name=f"ln_{nm}",
                                    tag="lnbc", bufs=4)
            xts = [pre.tile([128, H], bf16, name=f"xt{tt}", tag="xt", bufs=4)
                   for tt in range(4)]

            def layer_norm_to(dst, src, g_bc, b_bc, work):
                """dst = gamma*(src-mean)/(std_unbiased+EPS)+beta, bf16 2x."""
                stats = fsm.tile([128, 2, 6], f32, tag="stats", name="stats")
                for hf in range(2):
                    nc.vector.bn_stats(stats[:, hf, :],
                                       src[:, 512 * hf:512 * (hf + 1)])
                mv = fsm.tile([128, 2], f32, tag="mv", name="mv")
                nc.vector.bn_aggr(mv[:], stats[:])
                sd = fsm.tile([128, 1], f32, tag="sd", name="sd")
                nc.scalar.activation(sd[:], mv[:, 1:2], AF.Sqrt,
                                     scale=float(H) / (H - 1))
                nc.vector.tensor_scalar_add(sd[:], sd[:], EPS)
                rs = fsm.tile([128, 1], f32, tag="rs", name="rs")
                nc.vector.reciprocal(rs[:], sd[:])
                nc.vector.tensor_scalar(out=work[:], in0=src[:],
                                        scalar1=mv[:, 0:1], scalar2=rs[:],
                                        op0=ALU.subtract, op1=ALU.mult)
                nc.vector.tensor_mul(work[:], work[:], g_bc[:])
                nc.vector.tensor_add(dst[:], work[:], b_bc[:])

            # ---------------- attention scope ----------------
            actx = contextlib.ExitStack()
            with actx:
                qk = actx.enter_context(tc.tile_pool(name="qk", bufs=1))
                sb = actx.enter_context(tc.tile_pool(name="sb", bufs=3))
                small = actx.enter_context(tc.tile_pool(name="small", bufs=4))
                psn = actx.enter_context(
                    tc.tile_pool(name="psn", bufs=2, space="PSUM"))
                pss = actx.enter_context(
                    tc.tile_pool(name="pss", bufs=2, space="PSUM"))
                pso = actx.enter_context(
                    tc.tile_pool(name="pso", bufs=1, space="PSUM"))
                psm = actx.enter_context(
                    tc.tile_pool(name="psm", bufs=1, space="PSUM"))

                bs = const.tile([128, 2], f32)
                ident = const.tile([128, 128], f32)
                identb = const.tile([128, 128], bf16)

                # per-head score operands
                til_q, til_k, cr_q, cr_k = {}, {}, {}, {}
                for h in range(HPC):
                    til_q[h] = qk.tile([65, T], bf16, name=f"til_q{h}", tag="tq", bufs=HPC)
                    til_k[h] = qk.tile([65, T], bf16, name=f"til_k{h}", tag="tk", bufs=HPC)
                    cr_q[h] = qk.tile([128, T], bf16, name=f"cr_q{h}", tag="cq", bufs=HPC)
                    cr_k[h] = qk.tile([128, T], bf16, name=f"cr_k{h}", tag="ck", bufs=HPC)
                vn = []
                for kc in range(NKC):
                    vn.append(qk.tile([128, HPC, 65], bf16,
                                      name=f"vn{kc}", tag="vn", bufs=NKC))

                # ---- projection scope (xg/w freed before unit loop) ----
                pctx = contextlib.ExitStack()
                with pctx:
                    wpool = pctx.enter_context(tc.tile_pool(name="wpool", bufs=1))
                    xgp = pctx.enter_context(tc.tile_pool(name="xgp", bufs=2))
                    w_sb = {}
                    for w in ("wq", "wk", "wv"):
                        w_sb[w] = wpool.tile([128, KCH, 4 * HD], f32r, name=f"sb_{w}")
                    wk_r = ap["wk"].rearrange("(a p) c -> p a c", p=128)
                    xT_r = ap["xT"].rearrange("(a p) t -> p a t", p=128)
                    # critical-path first: wk half, xg0 half; xg rotates in
                    # 2 buffers, loaded one group ahead
                    xgs = {0: xgp.tile([128, KCH, 512], f32r, name="xg",
                                       tag="xg", bufs=2)}
                    nc.sync.dma_start(w_sb["wk"][:, 0:4, :], wk_r[:, 0:4, :])
                    nc.sync.dma_start(xgs[0][:, 0:4, :], xT_r[:, 0:4, 0:512])
                    nc.sync.dma_start(w_sb["wk"][:, 4:8, :], wk_r[:, 4:8, :])
                    nc.sync.dma_start(xgs[0][:, 4:8, :], xT_r[:, 4:8, 0:512])
                    nc.sync.dma_start(
                        w_sb["wq"][:], ap["wq"].rearrange("(a p) c -> p a c", p=128))
                    nc.sync.dma_start(
                        w_sb["wv"][:], ap["wv"].rearrange("(a p) c -> p a c", p=128))
                    nc.sync.dma_start(bs[:], ap["bsel"].partition_broadcast(128))

                    def load_xg(g):
                        xgs[g] = xgp.tile([128, KCH, 512], f32r, name="xg",
                                          tag="xg", bufs=2)
                        nc.sync.dma_start(
                            xgs[g][:], xT_r[:, :, 512 * g:512 * (g + 1)])
                    make_identity(nc, ident[:])
                    nc.vector.tensor_copy(identb[:], ident[:])
                    for h in range(HPC):
                        nc.gpsimd.memset(q8[h][:, 1, :], 0.0)
                        nc.gpsimd.memset(k8[h][:, 1, :], 0.0)
                        nc.gpsimd.memset(til_k[h][64:65, :], 1.0)
                    for kc in range(NKC):
                        nc.gpsimd.memset(vn[kc][:, :, 64:65], 1.0)

                    def emit_qk(name, til, cr, g, hp):
                        gsl = slice(512 * g, 512 * (g + 1))
                        p = pss.tile([128, 512], f32, tag="st", name="pqk")
                        for a in range(KCH):
                            nc.tensor.matmul(
                                p[:], w_sb[name][:, a, 128 * hp:128 * (hp + 1)],
                                xgs[g][:, a, :], start=(a == 0), stop=(a == KCH - 1))
                        for hl in range(2):
                            h = 2 * hp + hl
                            rows = slice(64 * hl, 64 * (hl + 1))
                            nc.scalar.activation(
                                til[h][0:64, gsl], p[rows, :], AF.Copy)
                            if name == "wq":
                                hi_rows, lo_rows = slice(0, 64), slice(64, 128)
                            else:
                                hi_rows, lo_rows = slice(64, 128), slice(0, 64)
                            nc.sync.dma_start(
                                cr[h][hi_rows, gsl], til[h][0:64, gsl])
                            nc.vector.scalar_tensor_tensor(
                                out=cr[h][lo_rows, gsl], in0=p[rows, :],
                                scalar=1.0, in1=til[h][0:64, gsl],
                                op0=ALU.mult, op1=ALU.subtract)

                    def emit_v(g, tt):
                        kc = 4 * g + tt
                        p = pss.tile([128, 4 * HD], f32, tag="st", name="pv")
                        for a in range(KCH):
                            nc.tensor.matmul(
                                p[:], xgs[g][:, a, 128 * tt:128 * (tt + 1)],
                                w_sb["wv"][:, a, :],
                                start=(a == 0), stop=(a == KCH - 1))
                        nc.scalar.activation(
                            vn[kc][:, :, 0:64],
                            p[:].rearrange("p (h d) -> p h d", h=HPC), AF.Copy)

                    for g in range(NG):
                        if g + 1 < NG:
                            load_xg(g + 1)
                        # interleave K/Q/V at psum granularity: heavy-consumer
                        # K/Q psums alternate with light V psums so the pss
                        # buffer pipeline never backs up
                        emit_qk("wk", til_k, cr_k, g, 0)
                        emit_qk("wq", til_q, cr_q, g, 0)
                        emit_v(g, 0)
                        emit_v(g, 1)
                        emit_qk("wk", til_k, cr_k, g, 1)
                        emit_qk("wq", til_q, cr_q, g, 1)
                        emit_v(g, 2)
                        emit_v(g, 3)
                        del xgs[g]

                # preload FFN-side inputs while PE is busy with attention
                for i, nm in enumerate(("g1", "b1", "g2", "b2")):
                    nc.sync.dma_start(
                        lnbc[nm][:], ap["lnw"][i, :].partition_broadcast(128))
                for tt in range(4):
                    nc.sync.dma_start(
                        xts[tt][:], ap["xres"][128 * tt:128 * (tt + 1), :])
                w1_r = ap["w1"].rearrange("(a p) f -> p a f", p=128)
                w1ts = {}
                for fb in range(2):  # prefetch first FFN1 chunks (bus is free)
                    w1ts[fb] = w1p.tile([128, KCH, 512], bf16, name="w1t")
                    nc.sync.dma_start(
                        w1ts[fb][:], w1_r[:, :, 512 * fb:512 * (fb + 1)])

                # ---- attention, software-pipelined over (g-major) units ----
                units = [(h, g) for g in range(NG) for h in range(HPC)]

                def stage_a1(h, g):
                    # fp8 DoubleRow natural-S + DVE max reduces
                    mstage = small.tile([128, 4], bf16, tag="mstage", name="mstage", bufs=2)
                    for qt in range(4):
                        qsl = slice(512 * g + 128 * qt, 512 * g + 128 * (qt + 1))
                        nm2 = small.tile([128, 2], bf16, tag="nm", name="nm")
                        for half in range(2):
                            sn = psn.tile([128, 1024], f32, name="sn")
                            for j in range(2):
                                ks = slice(1024 * half + 512 * j,
                                           1024 * half + 512 * (j + 1))
                                nc.tensor.matmul(
                                    sn[:, 512 * j:512 * (j + 1)],
                                    til_q[h][0:64, qsl], til_k[h][0:64, ks],
                                    start=True, stop=True)
                            nc.vector.tensor_reduce(
                                nm2[:, half:half + 1], sn[:],
                                axis=mybir.AxisListType.X, op=ALU.max)
                        # combine (+max; sign flipped in the stage_a2 rescale)
                        nc.vector.tensor_tensor(
                            mstage[:, qt:qt + 1], nm2[:, 0:1], nm2[:, 1:2],
                            ALU.max)
                    return mstage

                def stage_a2(h, g, mstage):
                    # emitted a period later so the PE transpose never waits on DVE
                    for qt in range(4):
                        qsl = slice(512 * g + 128 * qt, 512 * g + 128 * (qt + 1))
                        mt = psm.tile([1, 128], bf16, tag="mt", name="mt")
                        nc.tensor.transpose(mt[:], mstage[:, qt:qt + 1], identb[:])
                        # negate the +max into the m_hat_neg bias row
                        nc.vector.tensor_scalar_mul(til_q[h][64:65, qsl], mt[:], -1.0)

                def stage_b(h, g):
                    gsl = slice(512 * g, 512 * (g + 1))
                    # PV in natural layout: lhsT = P^T slice, rhs = V; the
                    # output lands token-major so no transposes are needed.
                    # All 4 qt accumulators share one PSUM bank: only the
                    # global first matmul uses start=True (the bank clear);
                    # the other qt groups' first writes overwrite their
                    # still-pending regions per the psum flag semantics.
                    o_acc = pso.tile([128, 4, 65], f32, tag="oa", name="o_acc")
                    pts = {}
                    PVLAG = 2

                    def pv(kc):
                        pt = pts.pop(kc)
                        for qt in range(4):
                            nc.tensor.matmul(
                                o_acc[:, qt, :], pt[:, 128 * qt:128 * (qt + 1)],
                                vn[kc][:, h, :],
                                start=(kc == 0 and qt == 0), stop=(kc == NKC - 1),
                                skip_group_check=True)

                    for kc in range(NKC):
                        ksl = slice(128 * kc, 128 * (kc + 1))
                        st = pss.tile([128, 512], f32, tag="st", name="st")
                        nc.tensor.matmul(st[:], til_k[h][0:65, ksl],
                                         til_q[h][0:65, gsl], start=True, stop=False)
                        nc.tensor.matmul(st[:], cr_k[h][:, ksl],
                                         cr_q[h][:, gsl], start=False, stop=True)
                        pt = sb.tile([128, 512], bf16, tag="pt", name="pt", bufs=6)
                        nc.scalar.activation(pt[:], st[:], AF.Exp)
                        pts[kc] = pt
                        if kc >= PVLAG:
                            pv(kc - PVLAG)
                    for kc in range(NKC - PVLAG, NKC):
                        pv(kc)
                    # scale by 1/denom; both batch halves get the same data,
                    # the receiver masks by bsel
                    ob = sb.tile([128, 4, HD], bf16, tag="ob", name="ob", bufs=2)
                    for qt in range(4):
                        rc = small.tile([128, 1], f32, tag="rc", name="rc")
                        nc.vector.reciprocal(rc[:], o_acc[:, qt, 64:65])
                        nc.vector.tensor_scalar_mul(
                            ob[:, qt, :], o_acc[:, qt, 0:64], rc[:])
                    nc.sync.dma_start(
                        a2a_in[g][0:4, :, 64 * h:64 * (h + 1)]
                        .rearrange("d p c -> p d c"), ob[:])
                    nc.sync.dma_start(
                        a2a_in[g][4:8, :, 64 * h:64 * (h + 1)]
                        .rearrange("d p c -> p d c"), ob[:])

                def ln1_quarter(q):
                    """Yields LN1 work in small chunks so it interleaves with
                    the next attention group instead of blocking the in-order
                    engine queues."""
                    at = fsb.tile([128, H], bf16, tag="ta", name="at", bufs=1)
                    bt = fsb.tile([128, H], bf16, tag="tb", name="bt", bufs=1)
                    nc.sync.dma_start(
                        at[:].rearrange("p (s c) -> p s c", s=4),
                        a2a_out[q][0:4, :, :].rearrange("s p c -> p s c"))
                    nc.sync.dma_start(
                        bt[:].rearrange("p (s c) -> p s c", s=4),
                        a2a_out[q][4:8, :, :].rearrange("s p c -> p s c"))
                    yield
                    af = fsb.tile([128, H], bf16, tag="tc", name="af", bufs=1)
                    nc.vector.tensor_scalar_mul(af[:], at[:], bs[:, 0:1])
                    nc.vector.scalar_tensor_tensor(
                        out=af[:], in0=bt[:], scalar=bs[:, 1:2], in1=af[:],
                        op0=ALU.mult, op1=ALU.add)
                    nc.vector.tensor_add(af[:], af[:], xts[q][:])
                    yield
                    wk_ = fsb.tile([128, H], bf16, tag="td", name="wk_", bufs=1)
                    layer_norm_to(out1[:, q, :], af, lnbc["g1"], lnbc["b1"], wk_)
                    yield
                    # transpose out1 quarter -> out1T (bf16 transposes)
                    for half in range(2):
                        for a in range(4 * half, 4 * half + 4):
                            tp = psm.tile([128, 128], bf16, tag="mt", name="tp")
                            nc.tensor.transpose(
                                tp[:], out1[:, q, 128 * a:128 * (a + 1)], identb[:])
                            nc.scalar.activation(
                                out1T[:, a, 128 * q:128 * (q + 1)], tp[:], AF.Copy)
                        yield

                def fire_a2a(q):
                    if sim_single:
                        nc.sync.dma_start(a2a_out[q][:], a2a_in[q][:])
                    else:
                        nc.gpsimd.collective_compute(
                            "AllToAll", ALU.bypass,
                            replica_groups=[list(range(NCORES))],
                            ins=[a2a_in[q].opt()], outs=[a2a_out[q].opt()])

                LOOKAHEAD = 3
                mstages, done_a2 = {}, set()
                ln1_gen = None
                for k in range(min(LOOKAHEAD, len(units))):
                    mstages[k] = stage_a1(*units[k])
                    stage_a2(*units[k], mstages.pop(k))
                    done_a2.add(k)
                for i, (h, g) in enumerate(units):
                    j = i + LOOKAHEAD
                    if j < len(units):
                        mstages[j] = stage_a1(*units[j])
                    j2 = i + LOOKAHEAD - 1
                    if j2 < len(units) and j2 not in done_a2:
                        stage_a2(*units[j2], mstages.pop(j2))
                        done_a2.add(j2)
                    stage_b(h, g)
                    # emit at most one LN1 chunk of the previous quarter per
                    # unit so its deps never park the in-order engine queues
                    if ln1_gen is not None:
                        if next(ln1_gen, "done") == "done":
                            ln1_gen = None
                    if h == HPC - 1:  # group g complete -> quarter a2a
                        fire_a2a(g)
                        while ln1_gen is not None:  # flush leftover chunks
                            if next(ln1_gen, "done") == "done":
                                ln1_gen = None
                        ln1_gen = ln1_quarter(g)
                if ln1_gen is not None:
                    for _ in ln1_gen:
                        pass

            # ---------------- FFN scope ----------------
            fctx = contextlib.ExitStack()
            with fctx:
                w2p = fctx.enter_context(tc.tile_pool(name="w2p", bufs=1))
                fhp = fctx.enter_context(tc.tile_pool(name="fhp", bufs=1))
                psf = fctx.enter_context(
                    tc.tile_pool(name="psf", bufs=3, space="PSUM"))
                ht = fhp.tile([128, FF // 128, 512], bf16, name="ht")

                w2_r = ap["w2"].rearrange("(a p) o -> p a o", p=128)
                w2ts = [w2p.tile([128, FF // 128, 512], bf16,
                                 name=f"w2t{oc}", tag="w2t", bufs=2)
                        for oc in range(2)]
                w2q = 0  # next w2 quarter-load to issue (8 x ~1MB chunks)

                def issue_w2_chunk():
                    nonlocal w2q
                    if w2q >= 8:
                        return
                    oc, sub = w2q // 4, w2q % 4
                    fsl = slice(8 * sub, 8 * (sub + 1))
                    nc.sync.dma_start(
                        w2ts[oc][:, fsl, :],
                        w2_r[:, fsl, 512 * oc:512 * (oc + 1)])
                    w2q += 1

                # FFN1: ht[f, t] = relu(W1^T x out1T), f-major; w2 quarter
                # loads are interleaved so no load hogs the DMA device.
                # The token dim splits 384/128 so the bulk of FFN1 only needs
                # LN1 quarters 0-2 and rolls straight out of attention while
                # the last quarter's a2a/LN1 chain completes.
                for fb in range(KCH):
                    if fb not in w1ts:
                        w1ts[fb] = w1p.tile([128, KCH, 512], bf16, name="w1t")
                        nc.sync.dma_start(
                            w1ts[fb][:], w1_r[:, :, 512 * fb:512 * (fb + 1)])
                    issue_w2_chunk()
                    w1t = w1ts.pop(fb)
                    for fq in range(4):  # 4 x 128 f-rows per block
                        ft = 4 * fb + fq
                        hpa = psf.tile([128, 384], f32, tag="hpa", name="hpa",
                                       bufs=3)
                        hpb = psf.tile([128, 128], f32, tag="hpb", name="hpb",
                                       bufs=2)
                        for a in range(KCH):
                            nc.tensor.matmul(
                                hpa[:], w1t[:, a, 128 * fq:128 * (fq + 1)],
                                out1T[:, a, 0:384],
                                start=(a == 0), stop=(a == KCH - 1))
                        nc.scalar.activation(ht[:, ft, 0:384], hpa[:], AF.Relu)
                        for a in range(KCH):
                            nc.tensor.matmul(
                                hpb[:], w1t[:, a, 128 * fq:128 * (fq + 1)],
                                out1T[:, a, 384:512],
                                start=(a == 0), stop=(a == KCH - 1))
                        nc.scalar.activation(ht[:, ft, 384:512], hpb[:], AF.Relu)
                issue_w2_chunk()
                issue_w2_chunk()

                # FFN2 token-major + fused LN2 tail
                f2 = fhp.tile([128, 4, H], bf16, name="f2")
                for tt in range(4):
                    for oc in range(2):
                        acc = psf.tile([128, 512], f32, tag="o2", name="o2acc",
                                       bufs=2)
                        for ft in range(FF // 128):
                            nc.tensor.matmul(
                                acc[:], ht[:, ft, 128 * tt:128 * (tt + 1)],
                                w2ts[oc][:, ft, :], start=(ft == 0),
                                stop=(ft == FF // 128 - 1))
                        nc.scalar.activation(
                            f2[:, tt, 512 * oc:512 * (oc + 1)], acc[:], AF.Copy)
                    h2 = fsb.tile([128, H], bf16, tag="ta", name="h2", bufs=1)
                    nc.vector.tensor_add(h2[:], out1[:, tt, :], f2[:, tt, :])
                    fin = fsb.tile([128, H], bf16, tag="tb", name="fin", bufs=1)
                    wk2 = fsb.tile([128, H], bf16, tag="tc", name="wk2", bufs=1)
                    layer_norm_to(fin, h2, lnbc["g2"], lnbc["b2"], wk2)
                    nc.sync.dma_start(out_ap[128 * tt:128 * (tt + 1), :], fin[:])

    nc.compile()
    if not sim_single:
        nc.m = get_hw_module(nc.m)
    return nc


_NC_CACHE = {}


def _get_program():
    if "nc" not in _NC_CACHE:
        _NC_CACHE["nc"] = _build_program()
    return _NC_CACHE["nc"]


def _prep_inputs(x, Wqkv, bqkv, W1, b1, W2, b2, gamma1, beta1, gamma2, beta2):
    """Host-side slicing/folding into per-core in_maps."""
    x = np.asarray(x, np.float32)
    Wqkv = np.asarray(Wqkv, np.float32)
    bqkv = np.asarray(bqkv, np.float32)
    d = np.arange(HD)
    hh = np.arange(NH)
    # qkv reshape in reference: [B,T,HD,3,NH] -> col = d*48 + k*16 + h
    cols = d[:, None, None] * (3 * NH) + np.arange(3)[None, :, None] * NH \
        + hh[None, None, :]
    Wq = Wqkv[:, cols[:, 0, :]] * (bqkv[cols[:, 0, :]] / np.sqrt(H))[None]
    Wk = Wqkv[:, cols[:, 1, :]] * bqkv[cols[:, 1, :]][None]
    Wv = Wqkv[:, cols[:, 2, :]] * bqkv[cols[:, 2, :]][None]
    # -> [H, HD, NH]; per-core head-major layout [H, 4*HD] (head-local major)
    Wq = np.transpose(Wq, (0, 2, 1))  # [H, NH, HD]
    Wk = np.transpose(Wk, (0, 2, 1))
    Wv = np.transpose(Wv, (0, 2, 1))
    W1e = (np.asarray(W1, np.float32) * np.asarray(b1, np.float32)[None]) \
        .astype(ml_dtypes.bfloat16)
    W2e = (np.asarray(W2, np.float32) * np.asarray(b2, np.float32)[None]) \
        .astype(ml_dtypes.bfloat16)
    lnw = np.stack([gamma1, beta1, gamma2, beta2]) \
        .astype(ml_dtypes.bfloat16)
    xT = [_round_mant(np.ascontiguousarray(x[b].T)) for b in range(B)]
    in_maps = []
    for c in range(NCORES):
        b, grp = c // 4, c % 4
        heads = slice(4 * grp, 4 * grp + 4)
        # strided token ownership: rows q*128+p <-> token 512q+128*grp+p
        xres = np.ascontiguousarray(
            x[b].reshape(4, 4, 128, H)[:, grp].reshape(TOK, H))
        in_maps.append({
            "xT": xT[b],
            "xres": xres.astype(ml_dtypes.bfloat16),
            "wq": _round_mant(Wq[:, heads, :].reshape(H, 4 * HD)),
            "wk": _round_mant(Wk[:, heads, :].reshape(H, 4 * HD)),
            "wv": _round_mant(Wv[:, heads, :].reshape(H, 4 * HD)),
            "w1": W1e, "w2": W2e, "lnw": lnw,
            "bsel": np.array([1.0, 0.0] if b == 0 else [0.0, 1.0], np.float32),
        })
    return in_maps


def kernel(x, Wqkv, bqkv, W1, b1, W2, b2, gamma1, beta1, gamma2, beta2,
           _trace=False):
    nc = _get_program()
    in_maps = _prep_inputs(x, Wqkv, bqkv, W1, b1, W2, b2,
                           gamma1, beta1, gamma2, beta2)
    res = run_bass_kernel_spmd(nc, in_maps, core_ids=list(range(NCORES)),
                               trace=_trace)
    out = np.empty((B, T, H), np.float32)
    for c in range(NCORES):
        b, grp = c // 4, c % 4
        out[b].reshape(4, 4, 128, H)[:, grp] = \
            np.asarray(res.results[c]["out"]).astype(np.float32) \
            .reshape(4, 128, H)
    if _trace:
        kernel.last_results = res
    return out


# revision 64
# speedup vs baseline: 1.1502x; 1.0549x over previous
"""TRN2 Bass kernel for nn_DecoderLayer_47175920779446.

Full decoder layer: qkv (mul-bias) -> 16-head attention -> +res -> LN ->
FFN(relu, mul-bias) -> +res -> LN, on x[2, 2048, 1024] fp32.

Sharding (8 cores): attention is sharded by (batch, 4 heads): core c handles
batch c//4, heads 4*(c%4)..4*(c%4)+3 over all 2048 tokens of its batch.
Four quarter-AllToAlls (bf16) reshard attention output to strided token
sharding: core c owns tokens {512q + 128*(c%4) + p} of batch c//4, so each
quarter fires right after its sender token-group finishes and LN1 overlaps
the remaining attention. Cross-batch blocks are zeroed at the sender (scaled
by the per-core bsel mask) so receivers just add both halves.

Precision: scores need ~fp32 accuracy (std ~256 feeding exp): q,k chain runs
fp32r projections (host-rounded 11-bit operands, fp32 accum), then an exact
bf16 hi/lo split with a 2-matmul scheme: S = qh*kh + m_hat (main, K=65 with
a fused bias row) plus [qh;ql]*[kl;kh] (cross, K=128). The row-max pass
(needs only ~+-50 accuracy) reuses the bf16 hi tiles. PV runs in natural
token-major layout (lhsT = P^T slices, 4 accumulators sharing one PSUM bank)
so no output transposes are needed. V/P/FFN run bf16; LN chains run bf16
(2x DVE); residual adds keep fp32 accumulation in PSUM.
"""
import contextlib
import numpy as np
import ml_dtypes

import concourse.bass as bass
import concourse.tile as tile
from concourse import bacc, mybir
from concourse.bass_utils import run_bass_kernel_spmd
from concourse.bass_interp import get_hw_module
from concourse.masks import make_identity

H, NH, HD, FF = 1024, 16, 64, 4096
B, T = 2, 2048
EPS = 1e-6
NCORES = 8
HPC = NH // 4          # 4 heads per core
TOK = (B * T) // NCORES  # 512 tokens per core
NKC = T // 128         # 16 key chunks
NG = T // 512          # 4 query groups
KCH = H // 128         # 8 contraction chunks for qkv
f32, f32r, bf16 = mybir.dt.float32, mybir.dt.float32r, mybir.dt.bfloat16
fp8 = mybir.dt.float8e4
AF = mybir.ActivationFunctionType
ALU = mybir.AluOpType
DR = mybir.MatmulPerfMode.DoubleRow


def _round_mant(x, bits=11):
    xi = np.ascontiguousarray(x, np.float32).view(np.int32)
    shift = 23 - bits
    bias = (1 << (shift - 1)) - 1 + ((xi >> shift) & 1)
    xi = (xi + bias) & ~((1 << shift) - 1)
    return xi.view(np.float32)


def _build_program(sim_single=False):
    nc = bacc.Bacc("TRN2", target_bir_lowering=False, debug=False,
                   num_devices=1 if sim_single else NCORES)
    ap = {}
    ap["xT"] = nc.dram_tensor("xT", [H, T], f32r, kind="ExternalInput").ap()
    ap["xres"] = nc.dram_tensor("xres", [TOK, H], bf16, kind="ExternalInput").ap()
    for w in ("wq", "wk", "wv"):
        ap[w] = nc.dram_tensor(w, [H, 4 * HD], f32r, kind="ExternalInput").ap()
    ap["w1"] = nc.dram_tensor("w1", [H, FF], bf16, kind="ExternalInput").ap()
    ap["w2"] = nc.dram_tensor("w2", [FF, H], bf16, kind="ExternalInput").ap()
    ap["lnw"] = nc.dram_tensor("lnw", [4, H], bf16, kind="ExternalInput").ap()
    ap["bsel"] = nc.dram_tensor("bsel", [2], f32, kind="ExternalInput").ap()
    out_ap = nc.dram_tensor("out", [TOK, H], bf16, kind="ExternalOutput").ap()

    with tile.TileContext(nc) as tc:
        ctx = contextlib.ExitStack()
        with ctx:
            const = ctx.enter_context(tc.tile_pool(name="const", bufs=1))
            dram = ctx.enter_context(tc.tile_pool(name="dram", bufs=1, space="DRAM"))
            pre = ctx.enter_context(tc.tile_pool(name="pre", bufs=1))
            w1p = ctx.enter_context(tc.tile_pool(name="w1p", bufs=3))
            fsb = ctx.enter_context(tc.tile_pool(name="fsb", bufs=1))
            fsm = ctx.enter_context(tc.tile_pool(name="fsm", bufs=4))
            o1p = ctx.enter_context(tc.tile_pool(name="o1p", bufs=1))

            # quarter a2a buffers: block d = [128 tokens, 256 head-cols]
            a2a_in = [dram.tile([NCORES, 128, 4 * HD], bf16, name=f"a2ai{q}")
                      for q in range(NG)]
            a2a_out = [dram.tile([NCORES, 128, 4 * HD], bf16, name=f"a2ao{q}")
                       for q in range(NG)]

            out1 = o1p.tile([128, 4, H], bf16, name="out1")      # natural
            out1T = o1p.tile([128, KCH, 512], bf16, name="out1T")  # transposed

            lnbc = {}
            for i, nm in enumerate(("g1", "b1", "g2", "b2")):
                lnbc[nm] = pre.tile([128, H], bf16, name=f"ln_{nm}",
                                    tag="lnbc", bufs=4)
            xts = [pre.tile([128, H], bf16, name=f"xt{tt}", tag="xt", bufs=4)
                   for tt in range(4)]

            def layer_norm_to(dst, src, g_bc, b_bc, work):
                """dst = gamma*(src-mean)/(std_unbiased+EPS)+beta, bf16 2x."""
                stats = fsm.tile([128, 2, 6], f32, tag="stats", name="stats")
                for hf in range(2):
                    nc.vector.bn_stats(stats[:, hf, :],
                                       src[:, 512 * hf:512 * (hf + 1)])
                mv = fsm.tile([128, 2], f32, tag="mv", name="mv")
                nc.vector.bn_aggr(mv[:], stats[:])
                sd = fsm.tile([128, 1], f32, tag="sd", name="sd")
                nc.scalar.activation(sd[:], mv[:, 1:2], AF.Sqrt,
                                     scale=float(H) / (H - 1))
                nc.vector.tensor_scalar_add(sd[:], sd[:], EPS)
                rs = fsm.tile([128, 1], f32, tag="rs", name="rs")
                nc.vector.reciprocal(rs[:], sd[:])
                nc.vector.tensor_scalar(out=work[:], in0=src[:],
                                        scalar1=mv[:, 0:1], scalar2=rs[:],
                                        op0=ALU.subtract, op1=ALU.mult)
                nc.vector.tensor_mul(work[:], work[:], g_bc[:])
                nc.vector.tensor_add(dst[:], work[:], b_bc[:])

            # ---------------- attention scope ----------------
            actx = contextlib.ExitStack()
            with actx:
                qk = actx.enter_context(tc.tile_pool(name="qk", bufs=1))
                sb = actx.enter_context(tc.tile_pool(name="sb", bufs=3))
                small = actx.enter_context(tc.tile_pool(name="small", bufs=4))
                psn = actx.enter_context(
                    tc.tile_pool(name="psn", bufs=2, space="PSUM"))
                pss = actx.enter_context(
                    tc.tile_pool(name="pss", bufs=2, space="PSUM"))
                pso = actx.enter_context(
                    tc.tile_pool(name="pso", bufs=1, space="PSUM"))
                psm = actx.enter_context(
                    tc.tile_pool(name="psm", bufs=1, space="PSUM"))

                bs = const.tile([128, 2], f32)
                ident = const.tile([128, 128], f32)
                identb = const.tile([128, 128], bf16)

                # per-head score operands
                til_q, til_k, cr_q, cr_k = {}, {}, {}, {}
                for h in range(HPC):
                    til_q[h] = qk.tile([65, T], bf16, name=f"til_q{h}", tag="tq", bufs=HPC)
                    til_k[h] = qk.tile([65, T], bf16, name=f"til_k{h}", tag="tk", bufs=HPC)
                    cr_q[h] = qk.tile([128, T], bf16, name=f"cr_q{h}", tag="cq", bufs=HPC)
                    cr_k[h] = qk.tile([128, T], bf16, name=f"cr_k{h}", tag="ck", bufs=HPC)
                vn = []
                for kc in range(NKC):
                    vn.append(qk.tile([128, HPC, 65], bf16,
                                      name=f"vn{kc}", tag="vn", bufs=NKC))

                # ---- projection scope (xg/w freed before unit loop) ----
                pctx = contextlib.ExitStack()
                with pctx:
                    wpool = pctx.enter_context(tc.tile_pool(name="wpool", bufs=1))
                    xgp = pctx.enter_context(tc.tile_pool(name="xgp", bufs=2))
                    w_sb = {}
                    for w in ("wq", "wk", "wv"):
                        w_sb[w] = wpool.tile([128, KCH, 4 * HD], f32r, name=f"sb_{w}")
                    wk_r = ap["wk"].rearrange("(a p) c -> p a c", p=128)
                    xT_r = ap["xT"].rearrange("(a p) t -> p a t", p=128)
                    # critical-path first: wk half, xg0 half; xg rotates in
                    # 2 buffers, loaded one group ahead
                    xgs = {0: xgp.tile([128, KCH, 512], f32r, name="xg",
                                       tag="xg", bufs=2)}
                    nc.sync.dma_start(w_sb["wk"][:, 0:4, :], wk_r[:, 0:4, :])
                    nc.sync.dma_start(xgs[0][:, 0:4, :], xT_r[:, 0:4, 0:512])
                    nc.sync.dma_start(w_sb["wk"][:, 4:8, :], wk_r[:, 4:8, :])
                    nc.sync.dma_start(xgs[0][:, 4:8, :], xT_r[:, 4:8, 0:512])
                    nc.sync.dma_start(
                        w_sb["wq"][:], ap["wq"].rearrange("(a p) c -> p a c", p=128))
                    nc.sync.dma_start(
                        w_sb["wv"][:], ap["wv"].rearrange("(a p) c -> p a c", p=128))
                    nc.sync.dma_start(bs[:], ap["bsel"].partition_broadcast(128))

                    def load_xg(g):
                        xgs[g] = xgp.tile([128, KCH, 512], f32r, name="xg",
                                          tag="xg", bufs=2)
                        nc.sync.dma_start(
                            xgs[g][:], xT_r[:, :, 512 * g:512 * (g + 1)])
                    make_identity(nc, ident[:])
                    nc.vector.tensor_copy(identb[:], ident[:])
                    for h in range(HPC):
                        nc.gpsimd.memset(q8[h][:, 1, :], 0.0)
                        nc.gpsimd.memset(k8[h][:, 1, :], 0.0)
                        nc.gpsimd.memset(til_k[h][64:65, :], 1.0)
                    for kc in range(NKC):
                        nc.gpsimd.memset(vn[kc][:, :, 64:65], 1.0)

                    def emit_qk(name, til, cr, g, hp):
                        gsl = slice(512 * g, 512 * (g + 1))
                        p = pss.tile([128, 512], f32, tag="st", name="pqk")
                        for a in range(KCH):
                            nc.tensor.matmul(
                                p[:], w_sb[name][:, a, 128 * hp:128 * (hp + 1)],
                                xgs[g][:, a, :], start=(a == 0), stop=(a == KCH - 1))
                        for hl in range(2):
                            h = 2 * hp + hl
                            rows = slice(64 * hl, 64 * (hl + 1))
                            nc.scalar.activation(
                                til[h][0:64, gsl], p[rows, :], AF.Copy)
                            if name == "wq":
                                hi_rows, lo_rows = slice(0, 64), slice(64, 128)
                            else:
                                hi_rows, lo_rows = slice(64, 128), slice(0, 64)
                            nc.sync.dma_start(
                                cr[h][hi_rows, gsl], til[h][0:64, gsl])
                            nc.vector.scalar_tensor_tensor(
                                out=cr[h][lo_rows, gsl], in0=p[rows, :],
                                scalar=1.0, in1=til[h][0:64, gsl],
                                op0=ALU.mult, op1=ALU.subtract)

                    def emit_v(g, tt):
                        kc = 4 * g + tt
                        p = pss.tile([128, 4 * HD], f32, tag="st", name="pv")
                        for a in range(KCH):
                            nc.tensor.matmul(
                                p[:], xgs[g][:, a, 128 * tt:128 * (tt + 1)],
                                w_sb["wv"][:, a, :],
                                start=(a == 0), stop=(a == KCH - 1))
                        nc.scalar.activation(
                            vn[kc][:, :, 0:64],
                            p[:].rearrange("p (h d) -> p h d", h=HPC), AF.Copy)

                    for g in range(NG):
                        if g + 1 < NG:
                            load_xg(g + 1)
                        # interleave K/Q/V at psum granularity: heavy-consumer
                        # K/Q psums alternate with light V psums so the pss
                        # buffer pipeline never backs up
                        emit_qk("wk", til_k, cr_k, g, 0)
                        emit_qk("wq", til_q, cr_q, g, 0)
                        emit_v(g, 0)
                        emit_v(g, 1)
                        emit_qk("wk", til_k, cr_k, g, 1)
                        emit_qk("wq", til_q, cr_q, g, 1)
                        emit_v(g, 2)
                        emit_v(g, 3)
                        del xgs[g]

                # preload FFN-side inputs while PE is busy with attention
                for i, nm in enumerate(("g1", "b1", "g2", "b2")):
                    nc.sync.dma_start(
                        lnbc[nm][:], ap["lnw"][i, :].partition_broadcast(128))
                for tt in range(4):
                    nc.sync.dma_start(
                        xts[tt][:], ap["xres"][128 * tt:128 * (tt + 1), :])
                w1_r = ap["w1"].rearrange("(a p) f -> p a f", p=128)
                w1ts = {}
                for fb in range(3):  # prefetch first FFN1 chunks (bus is free)
                    w1ts[fb] = w1p.tile([128, KCH, 512], bf16, name="w1t")
                    nc.sync.dma_start(
                        w1ts[fb][:], w1_r[:, :, 512 * fb:512 * (fb + 1)])

                # ---- attention, software-pipelined over (g-major) units ----
                units = [(h, g) for g in range(NG) for h in range(HPC)]

                def stage_a1(h, g):
                    # fp8 DoubleRow natural-S + DVE max reduces
                    mstage = small.tile([128, 4], bf16, tag="mstage", name="mstage", bufs=2)
                    for qt in range(4):
                        qsl = slice(512 * g + 128 * qt, 512 * g + 128 * (qt + 1))
                        nm2 = small.tile([128, 2], bf16, tag="nm", name="nm")
                        for half in range(2):
                            sn = psn.tile([128, 1024], f32, name="sn")
                            for j in range(2):
                                ks = slice(1024 * half + 512 * j,
                                           1024 * half + 512 * (j + 1))
                                nc.tensor.matmul(
                                    sn[:, 512 * j:512 * (j + 1)],
                                    til_q[h][0:64, qsl], til_k[h][0:64, ks],
                                    start=True, stop=True)
                            nc.vector.tensor_reduce(
                                nm2[:, half:half + 1], sn[:],
                                axis=mybir.AxisListType.X, op=ALU.max)
                        # combine (+max; sign flipped in the stage_a2 rescale)
                        nc.vector.tensor_tensor(
                            mstage[:, qt:qt + 1], nm2[:, 0:1], nm2[:, 1:2],
                            ALU.max)
                    return mstage

                def stage_a2(h, g, mstage):
                    # emitted a period later so the PE transpose never waits on DVE
                    for qt in range(4):
                        qsl = slice(512 * g + 128 * qt, 512 * g + 128 * (qt + 1))
                        mt = psm.tile([1, 128], bf16, tag="mt", name="mt")
                        nc.tensor.transpose(mt[:], mstage[:, qt:qt + 1], identb[:])
                        # negate the +max into the m_hat_neg bias row
                        nc.vector.tensor_scalar_mul(til_q[h][64:65, qsl], mt[:], -1.0)

                def stage_b(h, g):
                    gsl = slice(512 * g, 512 * (g + 1))
                    # PV in natural layout: lhsT = P^T slice, rhs = V; the
                    # output lands token-major so no transposes are needed.
                    # All 4 qt accumulators share one PSUM bank: only the
                    # global first matmul uses start=True (the bank clear);
                    # the other qt groups' first writes overwrite their
                    # still-pending regions per the psum flag semantics.
                    o_acc = pso.tile([128, 4, 65], f32, tag="oa", name="o_acc")
                    pts = {}
                    PVLAG = 4

                    def pv(kc):
                        pt = pts.pop(kc)
                        for qt in range(4):
                            nc.tensor.matmul(
                                o_acc[:, qt, :], pt[:, 128 * qt:128 * (qt + 1)],
                                vn[kc][:, h, :],
                                start=(kc == 0 and qt == 0), stop=(kc == NKC - 1),
                                skip_group_check=True)

                    for kc in range(NKC):
                        ksl = slice(128 * kc, 128 * (kc + 1))
                        st = pss.tile([128, 512], f32, tag="st", name="st")
                        nc.tensor.matmul(st[:], til_k[h][0:65, ksl],
                                         til_q[h][0:65, gsl], start=True, stop=False)
                        nc.tensor.matmul(st[:], cr_k[h][:, ksl],
                                         cr_q[h][:, gsl], start=False, stop=True)
                        pt = sb.tile([128, 512], bf16, tag="pt", name="pt", bufs=8)
                        nc.scalar.activation(pt[:], st[:], AF.Exp)
                        pts[kc] = pt
                        if kc >= PVLAG:
                            pv(kc - PVLAG)
                    for kc in range(NKC - PVLAG, NKC):
                        pv(kc)
                    # scale by 1/denom; both batch halves get the same data,
                    # the receiver masks by bsel
                    ob = sb.tile([128, 4, HD], bf16, tag="ob", name="ob", bufs=3)
                    for qt in range(4):
                        rc = small.tile([128, 1], f32, tag="rc", name="rc")
                        nc.vector.reciprocal(rc[:], o_acc[:, qt, 64:65])
                        nc.vector.tensor_scalar_mul(
                            ob[:, qt, :], o_acc[:, qt, 0:64], rc[:])
                    nc.sync.dma_start(
                        a2a_in[g][0:4, :, 64 * h:64 * (h + 1)]
                        .rearrange("d p c -> p d c"), ob[:])
                    nc.sync.dma_start(
                        a2a_in[g][4:8, :, 64 * h:64 * (h + 1)]
                        .rearrange("d p c -> p d c"), ob[:])

                def ln1_quarter(q):
                    """Yields LN1 work in small chunks so it interleaves with
                    the next attention group instead of blocking the in-order
                    engine queues."""
                    at = fsb.tile([128, H], bf16, tag="ta", name="at", bufs=1)
                    bt = fsb.tile([128, H], bf16, tag="tb", name="bt", bufs=1)
                    nc.sync.dma_start(
                        at[:].rearrange("p (s c) -> p s c", s=4),
                        a2a_out[q][0:4, :, :].rearrange("s p c -> p s c"))
                    nc.sync.dma_start(
                        bt[:].rearrange("p (s c) -> p s c", s=4),
                        a2a_out[q][4:8, :, :].rearrange("s p c -> p s c"))
                    yield
                    af = fsb.tile([128, H], bf16, tag="tc", name="af", bufs=1)
                    nc.vector.tensor_scalar_mul(af[:], at[:], bs[:, 0:1])
                    nc.vector.scalar_tensor_tensor(
                        out=af[:], in0=bt[:], scalar=bs[:, 1:2], in1=af[:],
                        op0=ALU.mult, op1=ALU.add)
                    nc.vector.tensor_add(af[:], af[:], xts[q][:])
                    yield
                    wk_ = fsb.tile([128, H], bf16, tag="td", name="wk_", bufs=1)
                    layer_norm_to(out1[:, q, :], af, lnbc["g1"], lnbc["b1"], wk_)
                    yield
                    # transpose out1 quarter -> out1T (bf16 transposes)
                    for half in range(2):
                        for a in range(4 * half, 4 * half + 4):
                            tp = psm.tile([128, 128], bf16, tag="mt", name="tp")
                            nc.tensor.transpose(
                                tp[:], out1[:, q, 128 * a:128 * (a + 1)], identb[:])
                            nc.scalar.activation(
                                out1T[:, a, 128 * q:128 * (q + 1)], tp[:], AF.Copy)
                        yield

                def fire_a2a(q):
                    if sim_single:
                        nc.sync.dma_start(a2a_out[q][:], a2a_in[q][:])
                    else:
                        nc.gpsimd.collective_compute(
                            "AllToAll", ALU.bypass,
                            replica_groups=[list(range(NCORES))],
                            ins=[a2a_in[q].opt()], outs=[a2a_out[q].opt()])

                LOOKAHEAD = 2
                mstages, done_a2 = {}, set()
                ln1_gen = None
                for k in range(min(LOOKAHEAD, len(units))):
                    mstages[k] = stage_a1(*units[k])
                    stage_a2(*units[k], mstages.pop(k))
                    done_a2.add(k)
                for i, (h, g) in enumerate(units):
                    j = i + LOOKAHEAD
                    if j < len(units):
                        mstages[j] = stage_a1(*units[j])
                    j2 = i + LOOKAHEAD - 1
                    if j2 < len(units) and j2 not in done_a2:
                        stage_a2(*units[j2], mstages.pop(j2))
                        done_a2.add(j2)
                    stage_b(h, g)
                    # emit at most one LN1 chunk of the previous quarter per
                    # unit so its deps never park the in-order engine queues
                    if ln1_gen is not None:
                        if next(ln1_gen, "done") == "done":
                            ln1_gen = None
                    if h == HPC - 1:  # group g complete -> quarter a2a
                        fire_a2a(g)
                        while ln1_gen is not None:  # flush leftover chunks
                            if next(ln1_gen, "done") == "done":
                                ln1_gen = None
                        ln1_gen = ln1_quarter(g)
                if ln1_gen is not None:
                    for _ in ln1_gen:
                        pass

            # ---------------- FFN scope ----------------
            fctx = contextlib.ExitStack()
            with fctx:
                w2p = fctx.enter_context(tc.tile_pool(name="w2p", bufs=1))
                fhp = fctx.enter_context(tc.tile_pool(name="fhp", bufs=1))
                psf = fctx.enter_context(
                    tc.tile_pool(name="psf", bufs=3, space="PSUM"))
                ht = fhp.tile([128, FF // 128, 512], bf16, name="ht")

                w2_r = ap["w2"].rearrange("(a p) o -> p a o", p=128)
                w2ts = [w2p.tile([128, FF // 128, 512], bf16,
                                 name=f"w2t{oc}", tag="w2t", bufs=2)
                        for oc in range(2)]
                w2q = 0  # next w2 quarter-load to issue (8 x ~1MB chunks)

                def issue_w2_chunk():
                    nonlocal w2q
                    if w2q >= 8:
                        return
                    oc, sub = w2q // 4, w2q % 4
                    fsl = slice(8 * sub, 8 * (sub + 1))
                    nc.sync.dma_start(
                        w2ts[oc][:, fsl, :],
                        w2_r[:, fsl, 512 * oc:512 * (oc + 1)])
                    w2q += 1

                # FFN1: ht[f, t] = relu(W1^T x out1T), f-major; w2 quarter
                # loads are interleaved so no load hogs the DMA device.
                # The token dim splits 384/128 so the bulk of FFN1 only needs
                # LN1 quarters 0-2 and rolls straight out of attention while
                # the last quarter's a2a/LN1 chain completes.
                for fb in range(KCH):
                    if fb not in w1ts:
                        w1ts[fb] = w1p.tile([128, KCH, 512], bf16, name="w1t")
                        nc.sync.dma_start(
                            w1ts[fb][:], w1_r[:, :, 512 * fb:512 * (fb + 1)])
                    issue_w2_chunk()
                    w1t = w1ts.pop(fb)
                    for fq in range(4):  # 4 x 128 f-rows per block
                        ft = 4 * fb + fq
                        hpa = psf.tile([128, 384], f32, tag="hpa", name="hpa",
                                       bufs=3)
                        hpb = psf.tile([128, 128], f32, tag="hpb", name="hpb",
                                       bufs=2)
                        for a in range(KCH):
                            nc.tensor.matmul(
                                hpa[:], w1t[:, a, 128 * fq:128 * (fq + 1)],
                                out1T[:, a, 0:384],
                                start=(a == 0), stop=(a == KCH - 1))
                        nc.scalar.activation(ht[:, ft, 0:384], hpa[:], AF.Relu)
                        for a in range(KCH):
                            nc.tensor.matmul(
                                hpb[:], w1t[:, a, 128 * fq:128 * (fq + 1)],
                                out1T[:, a, 384:512],
                                start=(a == 0), stop=(a == KCH - 1))
                        nc.scalar.activation(ht[:, ft, 384:512], hpb[:], AF.Relu)
                issue_w2_chunk()
                issue_w2_chunk()

                # FFN2 token-major + fused LN2 tail
                f2 = fhp.tile([128, 4, H], bf16, name="f2")
                for tt in range(4):
                    for oc in range(2):
                        acc = psf.tile([128, 512], f32, tag="o2", name="o2acc",
                                       bufs=2)
                        for ft in range(FF // 128):
                            nc.tensor.matmul(
                                acc[:], ht[:, ft, 128 * tt:128 * (tt + 1)],
                                w2ts[oc][:, ft, :], start=(ft == 0),
                                stop=(ft == FF // 128 - 1))
                        nc.scalar.activation(
                            f2[:, tt, 512 * oc:512 * (oc + 1)], acc[:], AF.Copy)
                    h2 = fsb.tile([128, H], bf16, tag="ta", name="h2", bufs=1)
                    nc.vector.tensor_add(h2[:], out1[:, tt, :], f2[:, tt, :])
                    fin = fsb.tile([128, H], bf16, tag="tb", name="fin", bufs=1)
                    wk2 = fsb.tile([128, H], bf16, tag="tc", name="wk2", bufs=1)
                    layer_norm_to(fin, h2, lnbc["g2"], lnbc["b2"], wk2)
                    nc.sync.dma_start(out_ap[128 * tt:128 * (tt + 1), :], fin[:])

    nc.compile()
    if not sim_single:
        nc.m = get_hw_module(nc.m)
    return nc


_NC_CACHE = {}


def _get_program():
    if "nc" not in _NC_CACHE:
        _NC_CACHE["nc"] = _build_program()
    return _NC_CACHE["nc"]


def _prep_inputs(x, Wqkv, bqkv, W1, b1, W2, b2, gamma1, beta1, gamma2, beta2):
    """Host-side slicing/folding into per-core in_maps."""
    x = np.asarray(x, np.float32)
    Wqkv = np.asarray(Wqkv, np.float32)
    bqkv = np.asarray(bqkv, np.float32)
    d = np.arange(HD)
    hh = np.arange(NH)
    # qkv reshape in reference: [B,T,HD,3,NH] -> col = d*48 + k*16 + h
    cols = d[:, None, None] * (3 * NH) + np.arange(3)[None, :, None] * NH \
        + hh[None, None, :]
    Wq = Wqkv[:, cols[:, 0, :]] * (bqkv[cols[:, 0, :]] / np.sqrt(H))[None]
    Wk = Wqkv[:, cols[:, 1, :]] * bqkv[cols[:, 1, :]][None]
    Wv = Wqkv[:, cols[:, 2, :]] * bqkv[cols[:, 2, :]][None]
    # -> [H, HD, NH]; per-core head-major layout [H, 4*HD] (head-local major)
    Wq = np.transpose(Wq, (0, 2, 1))  # [H, NH, HD]
    Wk = np.transpose(Wk, (0, 2, 1))
    Wv = np.transpose(Wv, (0, 2, 1))
    W1e = (np.asarray(W1, np.float32) * np.asarray(b1, np.float32)[None]) \
        .astype(ml_dtypes.bfloat16)
    W2e = (np.asarray(W2, np.float32) * np.asarray(b2, np.float32)[None]) \
        .astype(ml_dtypes.bfloat16)
    lnw = np.stack([gamma1, beta1, gamma2, beta2]) \
        .astype(ml_dtypes.bfloat16)
    xT = [_round_mant(np.ascontiguousarray(x[b].T)) for b in range(B)]
    in_maps = []
    for c in range(NCORES):
        b, grp = c // 4, c % 4
        heads = slice(4 * grp, 4 * grp + 4)
        # strided token ownership: rows q*128+p <-> token 512q+128*grp+p
        xres = np.ascontiguousarray(
            x[b].reshape(4, 4, 128, H)[:, grp].reshape(TOK, H))
        in_maps.append({
            "xT": xT[b],
            "xres": xres.astype(ml_dtypes.bfloat16),
            "wq": _round_mant(Wq[:, heads, :].reshape(H, 4 * HD)),
            "wk": _round_mant(Wk[:, heads, :].reshape(H, 4 * HD)),
            "wv": _round_mant(Wv[:, heads, :].reshape(H, 4 * HD)),
            "w1": W1e, "w2": W2e, "lnw": lnw,
            "bsel": np.array([1.0, 0.0] if b == 0 else [0.0, 1.0], np.float32),
        })
    return in_maps


def kernel(x, Wqkv, bqkv, W1, b1, W2, b2, gamma1, beta1, gamma2, beta2,
           _trace=False):
    nc = _get_program()
    in_maps = _prep_inputs(x, Wqkv, bqkv, W1, b1, W2, b2,
                           gamma1, beta1, gamma2, beta2)
    res = run_bass_kernel_spmd(nc, in_maps, core_ids=list(range(NCORES)),
                               trace=_trace)
    out = np.empty((B, T, H), np.float32)
    for c in range(NCORES):
        b, grp = c // 4, c % 4
        out[b].reshape(4, 4, 128, H)[:, grp] = \
            np.asarray(res.results[c]["out"]).astype(np.float32) \
            .reshape(4, 128, H)
    if _trace:
        kernel.last_results = res
    return out


# revision 73
# speedup vs baseline: 1.1525x; 1.0020x over previous
"""TRN2 Bass kernel for nn_DecoderLayer_47175920779446.

Full decoder layer: qkv (mul-bias) -> 16-head attention -> +res -> LN ->
FFN(relu, mul-bias) -> +res -> LN, on x[2, 2048, 1024] fp32.

Sharding (8 cores): attention is sharded by (batch, 4 heads): core c handles
batch c//4, heads 4*(c%4)..4*(c%4)+3 over all 2048 tokens of its batch.
Four quarter-AllToAlls (bf16) reshard attention output to strided token
sharding: core c owns tokens {512q + 128*(c%4) + p} of batch c//4, so each
quarter fires right after its sender token-group finishes and LN1 overlaps
the remaining attention. Cross-batch blocks are zeroed at the sender (scaled
by the per-core bsel mask) so receivers just add both halves.

Precision: scores need ~fp32 accuracy (std ~256 feeding exp): q,k chain runs
fp32r projections (host-rounded 11-bit operands, fp32 accum), then an exact
bf16 hi/lo split with a 2-matmul scheme: S = qh*kh + m_hat (main, K=65 with
a fused bias row) plus [qh;ql]*[kl;kh] (cross, K=128). The row-max pass
(needs only ~+-50 accuracy) reuses the bf16 hi tiles. PV runs in natural
token-major layout (lhsT = P^T slices, 4 accumulators sharing one PSUM bank)
so no output transposes are needed. V/P/FFN run bf16; LN chains run bf16
(2x DVE); residual adds keep fp32 accumulation in PSUM.
"""
import contextlib
import numpy as np
import ml_dtypes

import concourse.bass as bass
import concourse.tile as tile
from concourse import bacc, mybir
from concourse.bass_utils import run_bass_kernel_spmd
from concourse.bass_interp import get_hw_module
from concourse.masks import make_identity

H, NH, HD, FF = 1024, 16, 64, 4096
B, T = 2, 2048
EPS = 1e-6
NCORES = 8
HPC = NH // 4          # 4 heads per core
TOK = (B * T) // NCORES  # 512 tokens per core
NKC = T // 128         # 16 key chunks
NG = T // 512          # 4 query groups
KCH = H // 128         # 8 contraction chunks for qkv
f32, f32r, bf16 = mybir.dt.float32, mybir.dt.float32r, mybir.dt.bfloat16
fp8 = mybir.dt.float8e4
AF = mybir.ActivationFunctionType
ALU = mybir.AluOpType
DR = mybir.MatmulPerfMode.DoubleRow


def _round_mant(x, bits=11):
    xi = np.ascontiguousarray(x, np.float32).view(np.int32)
    shift = 23 - bits
    bias = (1 << (shift - 1)) - 1 + ((xi >> shift) & 1)
    xi = (xi + bias) & ~((1 << shift) - 1)
    return xi.view(np.float32)


def _build_program(sim_single=False):
    nc = bacc.Bacc("TRN2", target_bir_lowering=False, debug=False,
                   num_devices=1 if sim_single else NCORES)
    ap = {}
    ap["xT"] = nc.dram_tensor("xT", [H, T], f32r, kind="ExternalInput").ap()
    ap["xres"] = nc.dram_tensor("xres", [TOK, H], bf16, kind="ExternalInput").ap()
    for w in ("wq", "wk", "wv"):
        ap[w] = nc.dram_tensor(w, [H, 4 * HD], f32r, kind="ExternalInput").ap()
    ap["w1"] = nc.dram_tensor("w1", [H, FF], bf16, kind="ExternalInput").ap()
    ap["w2"] = nc.dram_tensor("w2", [FF, H], bf16, kind="ExternalInput").ap()
    ap["lnw"] = nc.dram_tensor("lnw", [4, H], bf16, kind="ExternalInput").ap()
    ap["bsel"] = nc.dram_tensor("bsel", [2], f32, kind="ExternalInput").ap()
    out_ap = nc.dram_tensor("out", [TOK, H], bf16, kind="ExternalOutput").ap()

    with tile.TileContext(nc) as tc:
        ctx = contextlib.ExitStack()
        with ctx:
            const = ctx.enter_context(tc.tile_pool(name="const", bufs=1))
            dram = ctx.enter_context(tc.tile_pool(name="dram", bufs=1, space="DRAM"))
            pre = ctx.enter_context(tc.tile_pool(name="pre", bufs=1))
            w1p = ctx.enter_context(tc.tile_pool(name="w1p", bufs=3))
            fsb = ctx.enter_context(tc.tile_pool(name="fsb", bufs=1))
            fsm = ctx.enter_context(tc.tile_pool(name="fsm", bufs=4))
            o1p = ctx.enter_context(tc.tile_pool(name="o1p", bufs=1))

            # quarter a2a buffers: block d = [128 tokens, 256 head-cols]
            a2a_in = [dram.tile([NCORES, 128, 4 * HD], bf16, name=f"a2ai{q}")
                      for q in range(NG)]
            a2a_out = [dram.tile([NCORES, 128, 4 * HD], bf16, name=f"a2ao{q}")
                       for q in range(NG)]

            out1 = o1p.tile([128, 4, H], bf16, name="out1")      # natural
            out1T = o1p.tile([128, KCH, 512], bf16, name="out1T")  # transposed

            lnbc = {}
            for i, nm in enumerate(("g1", "b1", "g2", "b2")):
                lnbc[nm] = pre.tile([128, H], bf16, name=f"ln_{nm}",
                                    tag="lnbc", bufs=4)
            xts = [pre.tile([128, H], bf16, name=f"xt{tt}", tag="xt", bufs=4)
                   for tt in range(4)]

            def layer_norm_to(dst, src, g_bc, b_bc, work):
                """dst = gamma*(src-mean)/(std_unbiased+EPS)+beta, bf16 2x."""
                stats = fsm.tile([128, 2, 6], f32, tag="stats", name="stats")
                for hf in range(2):
                    nc.vector.bn_stats(stats[:, hf, :],
                                       src[:, 512 * hf:512 * (hf + 1)])
                mv = fsm.tile([128, 2], f32, tag="mv", name="mv")
                nc.vector.bn_aggr(mv[:], stats[:])
                sd = fsm.tile([128, 1], f32, tag="sd", name="sd")
                nc.scalar.activation(sd[:], mv[:, 1:2], AF.Sqrt,
                                     scale=float(H) / (H - 1))
                nc.vector.tensor_scalar_add(sd[:], sd[:], EPS)
                rs = fsm.tile([128, 1], f32, tag="rs", name="rs")
                nc.vector.reciprocal(rs[:], sd[:])
                nc.vector.tensor_scalar(out=work[:], in0=src[:],
                                        scalar1=mv[:, 0:1], scalar2=rs[:],
                                        op0=ALU.subtract, op1=ALU.mult)
                nc.vector.tensor_mul(work[:], work[:], g_bc[:])
                nc.vector.tensor_add(dst[:], work[:], b_bc[:])

            # ---------------- attention scope ----------------
            actx = contextlib.ExitStack()
            with actx:
                qk = actx.enter_context(tc.tile_pool(name="qk", bufs=1))
                sb = actx.enter_context(tc.tile_pool(name="sb", bufs=3))
                small = actx.enter_context(tc.tile_pool(name="small", bufs=4))
                psn = actx.enter_context(
                    tc.tile_pool(name="psn", bufs=2, space="PSUM"))
                pss = actx.enter_context(
                    tc.tile_pool(name="pss", bufs=2, space="PSUM"))
                pso = actx.enter_context(
                    tc.tile_pool(name="pso", bufs=1, space="PSUM"))
                psm = actx.enter_context(
                    tc.tile_pool(name="psm", bufs=1, space="PSUM"))

                bs = const.tile([128, 2], f32)
                ident = const.tile([128, 128], f32)
                identb = const.tile([128, 128], bf16)

                # per-head score operands
                til_q, til_k, cr_q, cr_k = {}, {}, {}, {}
                for h in range(HPC):
                    til_q[h] = qk.tile([65, T], bf16, name=f"til_q{h}", tag="tq", bufs=HPC)
                    til_k[h] = qk.tile([65, T], bf16, name=f"til_k{h}", tag="tk", bufs=HPC)
                    cr_q[h] = qk.tile([128, T], bf16, name=f"cr_q{h}", tag="cq", bufs=HPC)
                    cr_k[h] = qk.tile([128, T], bf16, name=f"cr_k{h}", tag="ck", bufs=HPC)
                vn = []
                for kc in range(NKC):
                    vn.append(qk.tile([128, HPC, 65], bf16,
                                      name=f"vn{kc}", tag="vn", bufs=NKC))

                # ---- projection scope (xg/w freed before unit loop) ----
                pctx = contextlib.ExitStack()
                with pctx:
                    wpool = pctx.enter_context(tc.tile_pool(name="wpool", bufs=1))
                    xgp = pctx.enter_context(tc.tile_pool(name="xgp", bufs=2))
                    w_sb = {}
                    for w in ("wq", "wk", "wv"):
                        w_sb[w] = wpool.tile([128, KCH, 4 * HD], f32r, name=f"sb_{w}")
                    wk_r = ap["wk"].rearrange("(a p) c -> p a c", p=128)
                    xT_r = ap["xT"].rearrange("(a p) t -> p a t", p=128)
                    # critical-path first: wk half, xg0 half; xg rotates in
                    # 2 buffers, loaded one group ahead
                    xgs = {0: xgp.tile([128, KCH, 512], f32r, name="xg",
                                       tag="xg", bufs=2)}
                    nc.sync.dma_start(w_sb["wk"][:, 0:4, :], wk_r[:, 0:4, :])
                    nc.sync.dma_start(xgs[0][:, 0:4, :], xT_r[:, 0:4, 0:512])
                    nc.sync.dma_start(w_sb["wk"][:, 4:8, :], wk_r[:, 4:8, :])
                    nc.sync.dma_start(xgs[0][:, 4:8, :], xT_r[:, 4:8, 0:512])
                    nc.sync.dma_start(
                        w_sb["wq"][:], ap["wq"].rearrange("(a p) c -> p a c", p=128))
                    nc.sync.dma_start(
                        w_sb["wv"][:], ap["wv"].rearrange("(a p) c -> p a c", p=128))
                    nc.sync.dma_start(bs[:], ap["bsel"].partition_broadcast(128))

                    def load_xg(g):
                        xgs[g] = xgp.tile([128, KCH, 512], f32r, name="xg",
                                          tag="xg", bufs=2)
                        nc.sync.dma_start(
                            xgs[g][:], xT_r[:, :, 512 * g:512 * (g + 1)])
                    make_identity(nc, ident[:])
                    nc.vector.tensor_copy(identb[:], ident[:])
                    for h in range(HPC):
                        nc.gpsimd.memset(q8[h][:, 1, :], 0.0)
                        nc.gpsimd.memset(k8[h][:, 1, :], 0.0)
                        nc.gpsimd.memset(til_k[h][64:65, :], 1.0)
                    for kc in range(NKC):
                        nc.gpsimd.memset(vn[kc][:, :, 64:65], 1.0)

                    def emit_qk(name, til, cr, g, hp):
                        gsl = slice(512 * g, 512 * (g + 1))
                        p = pss.tile([128, 512], f32, tag="st", name="pqk")
                        for a in range(KCH):
                            nc.tensor.matmul(
                                p[:], w_sb[name][:, a, 128 * hp:128 * (hp + 1)],
                                xgs[g][:, a, :], start=(a == 0), stop=(a == KCH - 1))
                        for hl in range(2):
                            h = 2 * hp + hl
                            rows = slice(64 * hl, 64 * (hl + 1))
                            nc.scalar.activation(
                                til[h][0:64, gsl], p[rows, :], AF.Copy)
                            if name == "wq":
                                hi_rows, lo_rows = slice(0, 64), slice(64, 128)
                            else:
                                hi_rows, lo_rows = slice(64, 128), slice(0, 64)
                            nc.sync.dma_start(
                                cr[h][hi_rows, gsl], til[h][0:64, gsl])
                            nc.vector.scalar_tensor_tensor(
                                out=cr[h][lo_rows, gsl], in0=p[rows, :],
                                scalar=1.0, in1=til[h][0:64, gsl],
                                op0=ALU.mult, op1=ALU.subtract)

                    def emit_v(g, tt):
                        kc = 4 * g + tt
                        p = pss.tile([128, 4 * HD], f32, tag="st", name="pv")
                        for a in range(KCH):
                            nc.tensor.matmul(
                                p[:], xgs[g][:, a, 128 * tt:128 * (tt + 1)],
                                w_sb["wv"][:, a, :],
                                start=(a == 0), stop=(a == KCH - 1))
                        nc.scalar.activation(
                            vn[kc][:, :, 0:64],
                            p[:].rearrange("p (h d) -> p h d", h=HPC), AF.Copy)

                    for g in range(NG):
                        if g + 1 < NG:
                            load_xg(g + 1)
                        # interleave K/Q/V at psum granularity: heavy-consumer
                        # K/Q psums alternate with light V psums so the pss
                        # buffer pipeline never backs up
                        emit_qk("wk", til_k, cr_k, g, 0)
                        emit_qk("wq", til_q, cr_q, g, 0)
                        emit_v(g, 0)
                        emit_v(g, 1)
                        emit_qk("wk", til_k, cr_k, g, 1)
                        emit_qk("wq", til_q, cr_q, g, 1)
                        emit_v(g, 2)
                        emit_v(g, 3)
                        del xgs[g]

                # preload FFN-side inputs while PE is busy with attention
                for i, nm in enumerate(("g1", "b1", "g2", "b2")):
                    nc.sync.dma_start(
                        lnbc[nm][:], ap["lnw"][i, :].partition_broadcast(128))
                for tt in range(4):
                    nc.sync.dma_start(
                        xts[tt][:], ap["xres"][128 * tt:128 * (tt + 1), :])
                w1_r = ap["w1"].rearrange("(a p) f -> p a f", p=128)
                w1ts = {}
                for fb in range(3):  # prefetch first FFN1 chunks (bus is free)
                    w1ts[fb] = w1p.tile([128, KCH, 512], bf16, name="w1t")
                    nc.sync.dma_start(
                        w1ts[fb][:], w1_r[:, :, 512 * fb:512 * (fb + 1)])

                # ---- attention, software-pipelined over (g-major) units ----
                units = [(h, g) for g in range(NG) for h in range(HPC)]

                def stage_a1(h, g):
                    # fp8 DoubleRow natural-S + DVE max reduces
                    mstage = small.tile([128, 4], bf16, tag="mstage", name="mstage", bufs=2)
                    for qt in range(4):
                        qsl = slice(512 * g + 128 * qt, 512 * g + 128 * (qt + 1))
                        nm2 = small.tile([128, 2], bf16, tag="nm", name="nm")
                        for half in range(2):
                            sn = psn.tile([128, 1024], f32, name="sn")
                            for j in range(2):
                                ks = slice(1024 * half + 512 * j,
                                           1024 * half + 512 * (j + 1))
                                nc.tensor.matmul(
                                    sn[:, 512 * j:512 * (j + 1)],
                                    til_q[h][0:64, qsl], til_k[h][0:64, ks],
                                    start=True, stop=True)
                            nc.vector.tensor_reduce(
                                nm2[:, half:half + 1], sn[:],
                                axis=mybir.AxisListType.X, op=ALU.max,
                                negate=True)
                        # combine negated maxes: mstage holds m_hat_neg
                        nc.vector.tensor_tensor(
                            mstage[:, qt:qt + 1], nm2[:, 0:1], nm2[:, 1:2],
                            ALU.min)
                    return mstage

                def stage_a2(h, g, mstage):
                    # emitted a period later so the PE transpose never waits on DVE
                    for qt in range(4):
                        qsl = slice(512 * g + 128 * qt, 512 * g + 128 * (qt + 1))
                        mt = psm.tile([1, 128], bf16, tag="mt", name="mt")
                        nc.tensor.transpose(mt[:], mstage[:, qt:qt + 1], identb[:])
                        nc.vector.tensor_copy(til_q[h][64:65, qsl], mt[:])

                def stage_b(h, g):
                    gsl = slice(512 * g, 512 * (g + 1))
                    # PV in natural layout: lhsT = P^T slice, rhs = V; the
                    # output lands token-major so no transposes are needed.
                    # All 4 qt accumulators share one PSUM bank: only the
                    # global first matmul uses start=True (the bank clear);
                    # the other qt groups' first writes overwrite their
                    # still-pending regions per the psum flag semantics.
                    o_acc = pso.tile([128, 4, 65], f32, tag="oa", name="o_acc")
                    pts = {}
                    PVLAG = 4

                    def pv(kc):
                        pt = pts.pop(kc)
                        for qt in range(4):
                            nc.tensor.matmul(
                                o_acc[:, qt, :], pt[:, 128 * qt:128 * (qt + 1)],
                                vn[kc][:, h, :],
                                start=(kc == 0 and qt == 0), stop=(kc == NKC - 1),
                                skip_group_check=True)

                    for kc in range(NKC):
                        ksl = slice(128 * kc, 128 * (kc + 1))
                        st = pss.tile([128, 512], f32, tag="st", name="st")
                        nc.tensor.matmul(st[:], til_k[h][0:65, ksl],
                                         til_q[h][0:65, gsl], start=True, stop=False)
                        nc.tensor.matmul(st[:], cr_k[h][:, ksl],
                                         cr_q[h][:, gsl], start=False, stop=True)
                        pt = sb.tile([128, 512], bf16, tag="pt", name="pt", bufs=8)
                        nc.scalar.activation(pt[:], st[:], AF.Exp)
                        pts[kc] = pt
                        if kc >= PVLAG:
                            pv(kc - PVLAG)
                    for kc in range(NKC - PVLAG, NKC):
                        pv(kc)
                    # scale by 1/denom; both batch halves get the same data,
                    # the receiver masks by bsel
                    ob = sb.tile([128, 4, HD], bf16, tag="ob", name="ob", bufs=3)
                    for qt in range(4):
                        rc = small.tile([128, 1], f32, tag="rc", name="rc")
                        nc.vector.reciprocal(rc[:], o_acc[:, qt, 64:65])
                        nc.vector.tensor_scalar_mul(
                            ob[:, qt, :], o_acc[:, qt, 0:64], rc[:])
                    nc.sync.dma_start(
                        a2a_in[g][0:4, :, 64 * h:64 * (h + 1)]
                        .rearrange("d p c -> p d c"), ob[:])
                    nc.sync.dma_start(
                        a2a_in[g][4:8, :, 64 * h:64 * (h + 1)]
                        .rearrange("d p c -> p d c"), ob[:])

                def ln1_quarter(q):
                    """Yields LN1 work in small chunks so it interleaves with
                    the next attention group instead of blocking the in-order
                    engine queues."""
                    at = fsb.tile([128, H], bf16, tag="ta", name="at", bufs=1)
                    bt = fsb.tile([128, H], bf16, tag="tb", name="bt", bufs=1)
                    nc.sync.dma_start(
                        at[:].rearrange("p (s c) -> p s c", s=4),
                        a2a_out[q][0:4, :, :].rearrange("s p c -> p s c"))
                    nc.sync.dma_start(
                        bt[:].rearrange("p (s c) -> p s c", s=4),
                        a2a_out[q][4:8, :, :].rearrange("s p c -> p s c"))
                    yield
                    af = fsb.tile([128, H], bf16, tag="tc", name="af", bufs=1)
                    nc.vector.tensor_scalar_mul(af[:], at[:], bs[:, 0:1])
                    nc.vector.scalar_tensor_tensor(
                        out=af[:], in0=bt[:], scalar=bs[:, 1:2], in1=af[:],
                        op0=ALU.mult, op1=ALU.add)
                    nc.vector.tensor_add(af[:], af[:], xts[q][:])
                    yield
                    wk_ = fsb.tile([128, H], bf16, tag="td", name="wk_", bufs=1)
                    layer_norm_to(out1[:, q, :], af, lnbc["g1"], lnbc["b1"], wk_)
                    yield
                    # transpose out1 quarter -> out1T (bf16 transposes)
                    for half in range(2):
                        for a in range(4 * half, 4 * half + 4):
                            tp = psm.tile([128, 128], bf16, tag="mt", name="tp")
                            nc.tensor.transpose(
                                tp[:], out1[:, q, 128 * a:128 * (a + 1)], identb[:])
                            nc.scalar.activation(
                                out1T[:, a, 128 * q:128 * (q + 1)], tp[:], AF.Copy)
                        yield

                def fire_a2a(q):
                    if sim_single:
                        nc.sync.dma_start(a2a_out[q][:], a2a_in[q][:])
                    else:
                        nc.gpsimd.collective_compute(
                            "AllToAll", ALU.bypass,
                            replica_groups=[list(range(NCORES))],
                            ins=[a2a_in[q].opt()], outs=[a2a_out[q].opt()])

                LOOKAHEAD = 2
                mstages, done_a2 = {}, set()
                ln1_gen = None
                for k in range(min(LOOKAHEAD, len(units))):
                    mstages[k] = stage_a1(*units[k])
                    stage_a2(*units[k], mstages.pop(k))
                    done_a2.add(k)
                for i, (h, g) in enumerate(units):
                    j = i + LOOKAHEAD
                    if j < len(units):
                        mstages[j] = stage_a1(*units[j])
                    j2 = i + LOOKAHEAD - 1
                    if j2 < len(units) and j2 not in done_a2:
                        stage_a2(*units[j2], mstages.pop(j2))
                        done_a2.add(j2)
                    stage_b(h, g)
                    # emit at most one LN1 chunk of the previous quarter per
                    # unit so its deps never park the in-order engine queues
                    if ln1_gen is not None:
                        if next(ln1_gen, "done") == "done":
                            ln1_gen = None
                    if h == HPC - 1:  # group g complete -> quarter a2a
                        fire_a2a(g)
                        while ln1_gen is not None:  # flush leftover chunks
                            if next(ln1_gen, "done") == "done":
                                ln1_gen = None
                        ln1_gen = ln1_quarter(g)
                if ln1_gen is not None:
                    for _ in ln1_gen:
                        pass

            # ---------------- FFN scope ----------------
            fctx = contextlib.ExitStack()
            with fctx:
                w2p = fctx.enter_context(tc.tile_pool(name="w2p", bufs=1))
                fhp = fctx.enter_context(tc.tile_pool(name="fhp", bufs=1))
                psf = fctx.enter_context(
                    tc.tile_pool(name="psf", bufs=3, space="PSUM"))
                ht = fhp.tile([128, FF // 128, 512], bf16, name="ht")

                w2_r = ap["w2"].rearrange("(a p) o -> p a o", p=128)
                w2ts = [w2p.tile([128, FF // 128, 512], bf16,
                                 name=f"w2t{oc}", tag="w2t", bufs=2)
                        for oc in range(2)]
                w2q = 0  # next w2 quarter-load to issue (8 x ~1MB chunks)

                def issue_w2_chunk():
                    nonlocal w2q
                    if w2q >= 8:
                        return
                    oc, sub = w2q // 4, w2q % 4
                    fsl = slice(8 * sub, 8 * (sub + 1))
                    nc.sync.dma_start(
                        w2ts[oc][:, fsl, :],
                        w2_r[:, fsl, 512 * oc:512 * (oc + 1)])
                    w2q += 1

                # FFN1: ht[f, t] = relu(W1^T x out1T), f-major; w2 quarter
                # loads are interleaved so no load hogs the DMA device.
                # The token dim splits 384/128 so the bulk of FFN1 only needs
                # LN1 quarters 0-2 and rolls straight out of attention while
                # the last quarter's a2a/LN1 chain completes.
                for fb in range(KCH):
                    if fb not in w1ts:
                        w1ts[fb] = w1p.tile([128, KCH, 512], bf16, name="w1t")
                        nc.sync.dma_start(
                            w1ts[fb][:], w1_r[:, :, 512 * fb:512 * (fb + 1)])
                    issue_w2_chunk()
                    w1t = w1ts.pop(fb)
                    for fq in range(4):  # 4 x 128 f-rows per block
                        ft = 4 * fb + fq
                        hpa = psf.tile([128, 384], f32, tag="hpa", name="hpa",
                                       bufs=3)
                        hpb = psf.tile([128, 128], f32, tag="hpb", name="hpb",
                                       bufs=2)
                        for a in range(KCH):
                            nc.tensor.matmul(
                                hpa[:], w1t[:, a, 128 * fq:128 * (fq + 1)],
                                out1T[:, a, 0:384],
                                start=(a == 0), stop=(a == KCH - 1))
                        nc.scalar.activation(ht[:, ft, 0:384], hpa[:], AF.Relu)
                        for a in range(KCH):
                            nc.tensor.matmul(
                                hpb[:], w1t[:, a, 128 * fq:128 * (fq + 1)],
                                out1T[:, a, 384:512],
                                start=(a == 0), stop=(a == KCH - 1))
                        nc.scalar.activation(ht[:, ft, 384:512], hpb[:], AF.Relu)
                issue_w2_chunk()
                issue_w2_chunk()

                # FFN2 token-major + fused LN2 tail: the residual add and
                # bn_stats run per oc-half so they overlap the other half's
                # matmuls, leaving only the normalize chain on the tail
                f2 = fhp.tile([128, 4, H], bf16, name="f2")
                for tt in range(4):
                    h2 = fsb.tile([128, H], bf16, tag="ta", name="h2", bufs=1)
                    stats = fsm.tile([128, 2, 6], f32, tag="stats", name="stats")
                    for oc in range(2):
                        csl = slice(512 * oc, 512 * (oc + 1))
                        acc = psf.tile([128, 512], f32, tag="o2", name="o2acc",
                                       bufs=2)
                        for ft in range(FF // 128):
                            nc.tensor.matmul(
                                acc[:], ht[:, ft, 128 * tt:128 * (tt + 1)],
                                w2ts[oc][:, ft, :], start=(ft == 0),
                                stop=(ft == FF // 128 - 1))
                        nc.scalar.activation(
                            f2[:, tt, csl], acc[:], AF.Copy)
                        nc.vector.tensor_add(
                            h2[:, csl], out1[:, tt, csl], f2[:, tt, csl])
                        nc.vector.bn_stats(stats[:, oc, :], h2[:, csl])
                    mv = fsm.tile([128, 2], f32, tag="mv", name="mv")
                    nc.vector.bn_aggr(mv[:], stats[:])
                    sd = fsm.tile([128, 1], f32, tag="sd", name="sd")
                    nc.scalar.activation(sd[:], mv[:, 1:2], AF.Sqrt,
                                         scale=float(H) / (H - 1))
                    nc.vector.tensor_scalar_add(sd[:], sd[:], EPS)
                    rs = fsm.tile([128, 1], f32, tag="rs", name="rs")
                    nc.vector.reciprocal(rs[:], sd[:])
                    fin = fsb.tile([128, H], bf16, tag="tb", name="fin", bufs=1)
                    wk2 = fsb.tile([128, H], bf16, tag="tc", name="wk2", bufs=1)
                    nc.vector.tensor_scalar(out=wk2[:], in0=h2[:],
                                            scalar1=mv[:, 0:1], scalar2=rs[:],
                                            op0=ALU.subtract, op1=ALU.mult)
                    nc.vector.tensor_mul(wk2[:], wk2[:], lnbc["g2"][:])
                    nc.vector.tensor_add(fin[:], wk2[:], lnbc["b2"][:])
                    nc.sync.dma_start(out_ap[128 * tt:128 * (tt + 1), :], fin[:])

    nc.compile()
    if not sim_single:
        nc.m = get_hw_module(nc.m)
    return nc


_NC_CACHE = {}


def _get_program():
    if "nc" not in _NC_CACHE:
        _NC_CACHE["nc"] = _build_program()
    return _NC_CACHE["nc"]


def _prep_inputs(x, Wqkv, bqkv, W1, b1, W2, b2, gamma1, beta1, gamma2, beta2):
    """Host-side slicing/folding into per-core in_maps."""
    x = np.asarray(x, np.float32)
    Wqkv = np.asarray(Wqkv, np.float32)
    bqkv = np.asarray(bqkv, np.float32)
    d = np.arange(HD)
    hh = np.arange(NH)
    # qkv reshape in reference: [B,T,HD,3,NH] -> col = d*48 + k*16 + h
    cols = d[:, None, None] * (3 * NH) + np.arange(3)[None, :, None] * NH \
        + hh[None, None, :]
    Wq = Wqkv[:, cols[:, 0, :]] * (bqkv[cols[:, 0, :]] / np.sqrt(H))[None]
    Wk = Wqkv[:, cols[:, 1, :]] * bqkv[cols[:, 1, :]][None]
    Wv = Wqkv[:, cols[:, 2, :]] * bqkv[cols[:, 2, :]][None]
    # -> [H, HD, NH]; per-core head-major layout [H, 4*HD] (head-local major)
    Wq = np.transpose(Wq, (0, 2, 1))  # [H, NH, HD]
    Wk = np.transpose(Wk, (0, 2, 1))
    Wv = np.transpose(Wv, (0, 2, 1))
    W1e = (np.asarray(W1, np.float32) * np.asarray(b1, np.float32)[None]) \
        .astype(ml_dtypes.bfloat16)
    W2e = (np.asarray(W2, np.float32) * np.asarray(b2, np.float32)[None]) \
        .astype(ml_dtypes.bfloat16)
    lnw = np.stack([gamma1, beta1, gamma2, beta2]) \
        .astype(ml_dtypes.bfloat16)
    xT = [_round_mant(np.ascontiguousarray(x[b].T)) for b in range(B)]
    in_maps = []
    for c in range(NCORES):
        b, grp = c // 4, c % 4
        heads = slice(4 * grp, 4 * grp + 4)
        # strided token ownership: rows q*128+p <-> token 512q+128*grp+p
        xres = np.ascontiguousarray(
            x[b].reshape(4, 4, 128, H)[:, grp].reshape(TOK, H))
        in_maps.append({
            "xT": xT[b],
            "xres": xres.astype(ml_dtypes.bfloat16),
            "wq": _round_mant(Wq[:, heads, :].reshape(H, 4 * HD)),
            "wk": _round_mant(Wk[:, heads, :].reshape(H, 4 * HD)),
            "wv": _round_mant(Wv[:, heads, :].reshape(H, 4 * HD)),
            "w1": W1e, "w2": W2e, "lnw": lnw,
            "bsel": np.array([1.0, 0.0] if b == 0 else [0.0, 1.0], np.float32),
        })
    return in_maps


def kernel(x, Wqkv, bqkv, W1, b1, W2, b2, gamma1, beta1, gamma2, beta2,
           _trace=False):
    nc = _get_program()
    in_maps = _prep_inputs(x, Wqkv, bqkv, W1, b1, W2, b2,
                           gamma1, beta1, gamma2, beta2)
    res = run_bass_kernel_spmd(nc, in_maps, core_ids=list(range(NCORES)),
                               trace=_trace)
    out = np.empty((B, T, H), np.float32)
    for c in range(NCORES):
        b, grp = c // 4, c % 4
        out[b].reshape(4, 4, 128, H)[:, grp] = \
            np.asarray(res.results[c]["out"]).astype(np.float32) \
            .reshape(4, 128, H)
    if _trace:
        kernel.last_results = res
    return out
